# revision 1
# baseline (speedup 1.0000x reference)
"""Trainium2 Bass kernel for CrossAttention (B=2, N=M=2048, 16 heads x 64).

Sharding: batch x head-group parallel over 8 cores. Core c handles batch
c//4 and heads [4*(c%4), 4*(c%4)+4). Projection weights are column-split
(Wq/Wk/Wv) / row-split (Wo) per core; each core produces a partial
[2048, 1024] output (bf16) which the host sums per batch (4 partials).

V2 design (cost-model driven):
  - All DRAM inputs and SBUF matmul operands are bf16 (same 1 cycle/row
    PE speed as f32r, half the DMA bytes, ~5e-3 rel err total).
  - ACT engine runs ONLY the softmax Exp (131072 rows = the 133us floor);
    all DMAs/copies live on SP/DVE/Pool queues.
  - PE is the bottleneck (393216 matmul rows = 163.8us floor).  Emission
    keeps PE saturated: KT -> QT(qc0) -> per q-chunk/head-pair S/exp/O
    streams with "fill" matmuls (QT for later q-chunks, fused output
    projection of earlier q-chunks, V during the first chunk) interleaved
    one per m-tile to absorb the exp-vs-PE rate gap.
  - Normalization: ones column in V_aug yields denominators in PSUM row
    64; DVE reciprocal, SP DMA to partition 0, gpsimd partition_broadcast,
    DVE multiply (bf16 out).  PSUM: 2x s_t (2 banks each) + 3x o_t +
    1 fill bank = 8 banks.
"""

import numpy as np
import ml_dtypes
from contextlib import ExitStack

import concourse.tile as tile
from concourse import bacc, mybir
from concourse.bass_utils import run_bass_kernel_spmd

B, N, M, C = 2, 2048, 2048, 1024
HEADS, D = 16, 64
HPC = 4            # heads per core
IC = HPC * D       # 256 inner dims per core
SCALE = D ** -0.5
NCORES = 8
KT_TILES = C // 128   # 8 contraction tiles for projections
P = 128
MT = M // P           # 16 m tiles
QC = 512
NQC = N // QC         # 4 q chunks
f32 = mybir.dt.float32
bf16 = mybir.dt.bfloat16

_CACHE = {}


def _body(nc, tc, ctx, xd, ctxd, wq, wk, wv, wo, out, opt=None):
    opt = opt or {}
    ES_BUFS = opt.get("es_bufs", 4)
    O_BUFS = opt.get("o_bufs", 2)
    F_BUFS = opt.get("f_bufs", 2)
    BCAST64 = opt.get("bcast64", False)
    GP_AOPACK = opt.get("gp_aopack", True)
    ONES0 = opt.get("ones0", True)

    const = ctx.enter_context(tc.tile_pool(name="const", bufs=1))
    wq_sb = const.tile([P, KT_TILES, IC], bf16, tag="wq")
    wk_sb = const.tile([P, KT_TILES, IC], bf16, tag="wk")
    wv_sb = const.tile([P, KT_TILES, IC], bf16, tag="wv")
    wo_sb = const.tile([P, 2, C], bf16, tag="wo")
    ctx_sb = const.tile([P, KT_TILES, M], bf16, tag="ctx")
    x_sb = const.tile([P, KT_TILES, N], bf16, tag="x")
    kt_sb = [const.tile([P, M], bf16, tag=f"kt{j}", name=f"kt{j}") for j in range(2)]
    qt_sb = [const.tile([P, N], bf16, tag=f"qt{j}", name=f"qt{j}") for j in range(2)]
    v_sb = const.tile([P, MT, HPC, D + 1], bf16, tag="v")
    ao_sb = [const.tile([P, N], bf16, tag=f"ao{j}", name=f"ao{j}") for j in range(2)]
    ones_sb = const.tile([P, 1], f32, tag="ones")

    es_pool = ctx.enter_context(tc.tile_pool(name="es", bufs=ES_BUFS))
    r_pool = ctx.enter_context(tc.tile_pool(name="rp", bufs=2))
    rb_pool = ctx.enter_context(tc.tile_pool(name="rbp", bufs=2))
    oc_pool = ctx.enter_context(tc.tile_pool(name="ocp", bufs=4))
    aot_pool = ctx.enter_context(tc.tile_pool(name="aot", bufs=2))
    outst_pool = ctx.enter_context(tc.tile_pool(name="outst", bufs=4))

    junk_sb = const.tile([P, QC], bf16, tag="junk")

    # junk tile first on DVE so PE warmup matmuls can start ~1us in
    nc.vector.memset(junk_sb[:], 0.0)
    # ones column of V_aug (f32 memset + broadcast-copy cast to bf16)
    nc.vector.memset(ones_sb[:], 1.0)
    nc.vector.tensor_copy(
        v_sb[:, :, :, D:D + 1],
        ones_sb[:, 0:1].to_broadcast((P, MT, HPC, 1)),
    )

    def emit_junk(pool, n, tag="warm"):
        # PE keep-warm matmuls (nothing reads the result): bridge idle
        # windows so the pstate ramp does not reset.
        jp = pool.tile([P, QC], f32, tag=tag, name=f"junk_{tag}")
        for _ in range(n):
            nc.tensor.matmul(
                jp[:], junk_sb[0:P, 0:P], junk_sb[:, 0:QC],
                start=True, stop=True,
            )

    # ---- input DMAs ----
    # ACT: weights + x qc0 (all done before the first exp); SP: ctx + x rest
    # first wk k-tile + ctx k0 quarters split so the first KT matmul starts early
    for k in range(KT_TILES):
        nc.scalar.dma_start(wq_sb[:, k, :], wq[:, k, :])
        nc.scalar.dma_start(x_sb[:, k, 0:QC], xd[:, k, 0:QC])
        nc.scalar.dma_start(wk_sb[:, k, :], wk[:, k, :])
    for k in range(KT_TILES):
        eng = nc.sync if k % 2 == 0 else nc.gpsimd
        eng.dma_start(ctx_sb[:, k, :], ctxd[:, k, :])
    nc.scalar.dma_start(wv_sb[:], wv[:])
    nc.scalar.dma_start(wo_sb[:], wo[:])
    for k in range(KT_TILES):
        nc.sync.dma_start(x_sb[:, k, QC:N], xd[:, k, QC:N])

    def emit_V(m, pool):
        vt = pool.tile([P, QC], f32, tag=pool_tag[id(pool)], name=f"vt{m}")
        for k in range(KT_TILES):
            nc.tensor.matmul(
                vt[:, 0:IC],
                ctx_sb[:, k, m * P:(m + 1) * P],
                wv_sb[:, k, :],
                start=(k == 0), stop=(k == KT_TILES - 1),
            )
        nc.vector.tensor_copy(
            v_sb[:, m, :, 0:D],
            vt[:, 0:IC].rearrange("p (h d) -> p h d", d=D),
        )

    pool_tag = {}

    # ---- PE warmup, then projections: KT (8 psum banks, k-outer), QT qc0 ----
    with tc.tile_pool(name="warm_ps", bufs=1, space="PSUM") as wps:
        emit_junk(wps, 6, tag="warm0")

    with tc.tile_pool(name="pp8", bufs=8, space="PSUM") as pps:
        pool_tag[id(pps)] = "pp"
        # QT qc0 (both j) and KT j0 interleaved per k-tile, tracking the
        # per-k weight/x/ctx DMA arrival order.
        qp = [pps.tile([P, QC], f32, tag="pp", name=f"qp{j}") for j in range(2)]
        kp0 = [pps.tile([P, QC], f32, tag="pp", name=f"kp0_{qc}") for qc in range(4)]
        for k in range(KT_TILES):
            for j in range(2):
                nc.tensor.matmul(
                    qp[j][:],
                    wq_sb[:, k, j * P:(j + 1) * P],
                    x_sb[:, k, 0:QC],
                    start=(k == 0), stop=(k == KT_TILES - 1),
                )
            for qc in range(4):
                nc.tensor.matmul(
                    kp0[qc][:],
                    wk_sb[:, k, 0:P],
                    ctx_sb[:, k, qc * QC:(qc + 1) * QC],
                    start=(k == 0), stop=(k == KT_TILES - 1),
                )
        nc.vector.tensor_copy(qt_sb[0][:, 0:QC], qp[0][:])
        nc.vector.tensor_copy(qt_sb[1][:, 0:QC], qp[1][:])
        for qc in range(4):
            nc.vector.tensor_copy(kt_sb[0][:, qc * QC:(qc + 1) * QC], kp0[qc][:])
        # KT j1 qc-major (ctx fully resident by now); last two chunks reuse
        # the QT banks, drained by the qt copies above.
        for qc in range(4):
            kp1 = pps.tile([P, QC], f32, tag="pp", name=f"kp1_{qc}")
            for k in range(KT_TILES):
                nc.tensor.matmul(
                    kp1[:],
                    wk_sb[:, k, P:2 * P],
                    ctx_sb[:, k, qc * QC:(qc + 1) * QC],
                    start=(k == 0), stop=(k == KT_TILES - 1),
                )
            nc.vector.tensor_copy(kt_sb[1][:, qc * QC:(qc + 1) * QC], kp1[:])
        # V[0]: covers the kt j1 copies + attention pool transition
        emit_V(0, pps)

    # ---- attention with interleaved fill work ----
    with ExitStack() as attn_ctx:
        sps = attn_ctx.enter_context(tc.tile_pool(name="s_ps", bufs=2, space="PSUM"))
        ops = attn_ctx.enter_context(tc.tile_pool(name="o_ps", bufs=O_BUFS, space="PSUM"))
        fps = attn_ctx.enter_context(tc.tile_pool(name="f_ps", bufs=F_BUFS, space="PSUM"))
        pool_tag[id(fps)] = "f"

        # QT fill state: one matmul per fill slot, j-serial per q-chunk
        qt_state = {}

        def emit_QT(qcn, slot):
            j, k = slot // KT_TILES, slot % KT_TILES
            if k == 0:
                qt_state[(qcn, j)] = fps.tile([P, QC], f32, tag="f", name=f"qf{qcn}_{j}")
            qtile = qt_state[(qcn, j)]
            nc.tensor.matmul(
                qtile[:],
                wq_sb[:, k, j * P:(j + 1) * P],
                x_sb[:, k, qcn * QC:(qcn + 1) * QC],
                start=(k == 0), stop=(k == KT_TILES - 1),
            )
            if k == KT_TILES - 1:
                nc.vector.tensor_copy(qt_sb[j][:, qcn * QC:(qcn + 1) * QC], qtile[:])
                del qt_state[(qcn, j)]

        # final projection fill: slot -> (nt, ec, j)
        fin_state = {}

        def emit_FIN(qcn, slot, pool):
            nt = qcn * 4 + slot // 4
            ec = (slot // 2) % 2
            j = slot % 2
            if j == 0:
                fin_state["ft"] = pool.tile(
                    [P, QC], f32, tag=pool_tag[id(pool)], name=f"ft{nt}_{ec}")
            ft = fin_state["ft"]
            nc.tensor.matmul(
                ft[:],
                ao_sb[j][:, nt * P:(nt + 1) * P],
                wo_sb[:, j, ec * QC:(ec + 1) * QC],
                start=(j == 0), stop=(j == 1),
            )
            if j == 0 and ec == 0:
                fin_state["ostg"] = outst_pool.tile([P, C], bf16, tag="ostg",
                                                    name=f"og{nt}")
            if j == 1:
                nc.vector.tensor_copy(fin_state["ostg"][:, ec * QC:(ec + 1) * QC], ft[:])
                if ec == 1:
                    deng = nc.sync if nt % 2 == 0 else nc.gpsimd
                    deng.dma_start(out[nt * P:(nt + 1) * P, :], fin_state["ostg"][:])

        # fill plan per (qc, p) segment
        def fill(qc, p, mt):
            if qc == 0 and p == 0:
                if mt + 1 < MT:
                    emit_V(mt + 1, fps)
            elif qc == 0 and p == 1:
                emit_QT(1, mt)
            elif qc == 1 and p == 0:
                emit_QT(2, mt)
            elif qc == 1 and p == 1:
                emit_FIN(0, mt, fps)
            elif qc == 2 and p == 0:
                emit_QT(3, mt)
            elif qc == 2 and p == 1:
                emit_FIN(1, mt, fps)
            elif qc == 3 and p == 0:
                # start 2 m-tiles late: normalize(qc2,p1) is still in flight
                if 2 <= mt <= 13:
                    emit_FIN(2, mt - 2, fps)
                elif mt >= 14:
                    emit_FIN(2, 2 * mt - 16, fps)
                    emit_FIN(2, 2 * mt - 15, fps)
            # (3,1): no fill available

        def normalize(qc, p, o_ts):
            # Chain (hh1 first; it gates the fused output projection):
            #   DVE: recip1 (straight from PSUM), oc1 copy, mul1, recip0, mul0
            #   Pool: bcast1, oc0 copy, bcast0
            # o banks freed by {recip, oc copy}; ao written by muls/pack-DMA.
            q0 = qc * QC
            rs, rbs, ocs = {}, {}, {}
            for hh in (1, 0):
                rs[hh] = r_pool.tile([P, QC], f32, tag="r", name=f"r{qc}_{p}_{hh}")
                rbs[hh] = rb_pool.tile([P, QC], f32, tag="rb", name=f"rb{qc}_{p}_{hh}")
                ocs[hh] = oc_pool.tile([P, QC], f32, tag="oc", name=f"oc{qc}_{p}_{hh}")

            def bcast(hh):
                if BCAST64:
                    nc.gpsimd.partition_broadcast(rbs[hh][0:D, :], rs[hh][64:65, :])
                else:
                    nc.sync.dma_start(rs[hh][0:1, :], rs[hh][64:65, :])
                    nc.gpsimd.partition_broadcast(rbs[hh][0:D, :], rs[hh][0:1, :])

            last = (qc == NQC - 1 and p == 1)
            if last:
                # recips straight from PSUM (bank release is moot here) so
                # the r DMAs start immediately; oc copies run on ACT behind
                # the final exp, in parallel.
                nc.vector.reciprocal(rs[1][64:65, :], o_ts[1][D:D + 1, :])
                nc.vector.reciprocal(rs[0][64:65, :], o_ts[0][D:D + 1, :])
                nc.scalar.copy(ocs[1][0:D + 1, :], o_ts[1][0:D + 1, :])
                nc.scalar.copy(ocs[0][0:D + 1, :], o_ts[0][0:D + 1, :])
                nc.sync.dma_start(rs[1][0:1, :], rs[1][64:65, :])
                nc.gpsimd.dma_start(rs[0][0:1, :], rs[0][64:65, :])
            else:
                nc.vector.tensor_copy(ocs[1][0:D + 1, :], o_ts[1][0:D + 1, :])
                nc.vector.tensor_copy(ocs[0][0:D + 1, :], o_ts[0][0:D + 1, :])
                nc.vector.reciprocal(rs[1][64:65, :], ocs[1][D:D + 1, :])
                nc.vector.reciprocal(rs[0][64:65, :], ocs[0][D:D + 1, :])
                nc.sync.dma_start(rs[1][0:1, :], rs[1][64:65, :])
                nc.gpsimd.dma_start(rs[0][0:1, :], rs[0][64:65, :])
            nc.gpsimd.partition_broadcast(rbs[1][0:D, :], rs[1][0:1, :])
            nc.gpsimd.partition_broadcast(rbs[0][0:D, :], rs[0][0:1, :])
            nc.gpsimd.tensor_mul(
                ao_sb[p][64:P, q0:q0 + QC], ocs[1][0:D, :], rbs[1][0:D, :]
            )
            nc.vector.tensor_mul(
                ao_sb[p][0:D, q0:q0 + QC], ocs[0][0:D, :], rbs[0][0:D, :]
            )

        for qc in range(NQC):
            q0 = qc * QC
            for p in range(2):
                o_ts = [ops.tile([P, QC], f32, tag="o", name=f"o{qc}_{p}_{i}")
                        for i in range(2)]
                if qc == 0 and p == 0:
                    pass  # V[0], V[1] were emitted in the projection scope
                for mt in range(MT):
                    s_t = sps.tile([P, 2 * QC], f32, tag="s", name=f"s{qc}_{p}_{mt}")
                    for hh in range(2):
                        pb = hh * 64
                        nc.tensor.matmul(
                            s_t[:, hh * QC:(hh + 1) * QC],
                            kt_sb[p][pb:pb + 64, mt * P:(mt + 1) * P],
                            qt_sb[p][pb:pb + 64, q0:q0 + QC],
                            start=True, stop=True,
                        )
                    es = es_pool.tile([P, 2 * QC], bf16, tag="es", name=f"es{qc}_{p}_{mt}")
                    nc.scalar.activation(
                        es[:], s_t[:],
                        mybir.ActivationFunctionType.Exp, scale=SCALE,
                    )
                    fill(qc, p, mt)
                    for hh in range(2):
                        h = 2 * p + hh
                        nc.tensor.matmul(
                            o_ts[hh][0:D + 1, :],
                            v_sb[:, mt, h, :],
                            es[:, hh * QC:(hh + 1) * QC],
                            start=(mt == 0), stop=(mt == MT - 1),
                        )
                normalize(qc, p, o_ts)

    # ---- tail: final projection for qc3 ----
    # ao p0 is ready well before ao p1 (its normalize ends the kernel), so:
    # j0 accumulation steps first (4 open banks), junk bridge keeps the PE
    # pstate warm while normalize(qc3,p1) completes, then the j1 steps.
    with (
        tc.tile_pool(name="tail_ps", bufs=7, space="PSUM") as tps,
        tc.tile_pool(name="tailj_ps", bufs=1, space="PSUM") as tjp,
    ):
        fts = {}
        ostgs = {}

        def fin3_mm(i, j):
            nt = 12 + i // 2
            ec = i % 2
            if j == 0:
                fts[i] = tps.tile([P, QC], f32, tag="tf", name=f"tf{i}")
            nc.tensor.matmul(
                fts[i][:],
                ao_sb[j][:, nt * P:(nt + 1) * P],
                wo_sb[:, j, ec * QC:(ec + 1) * QC],
                start=(j == 0), stop=(j == 1),
            )
            if j == 0 and ec == 0:
                ostgs[nt] = outst_pool.tile([P, C], bf16, tag="ostg", name=f"og{nt}")
            if j == 1:
                ceng = nc.scalar if ec == 0 else nc.vector
                if ec == 0:
                    nc.scalar.copy(ostgs[nt][:, ec * QC:(ec + 1) * QC], fts[i][:])
                else:
                    nc.vector.tensor_copy(ostgs[nt][:, ec * QC:(ec + 1) * QC], fts[i][:])
                deng = nc.sync if ec == 1 else nc.gpsimd
                deng.dma_start(out[nt * P:(nt + 1) * P, ec * QC:(ec + 1) * QC],
                               ostgs[nt][:, ec * QC:(ec + 1) * QC])

        for i in range(4):
            fin3_mm(i, 0)
        emit_junk(tjp, opt.get("tail_junk", 14), tag="warm1")
        for i in range(4):
            fin3_mm(i, 1)
        for i in range(4, 8):
            fin3_mm(i, 0)
        for i in (6, 7, 4, 5):   # last row group first so its copy+DMA drain early
            fin3_mm(i, 1)


def _build(reps=1, opt=None):
    key = (reps, tuple(sorted((opt or {}).items())))
    if key in _CACHE:
        return _CACHE[key]
    nc = bacc.Bacc("TRN2", target_bir_lowering=False, debug=False)
    xd = nc.dram_tensor("xd", [P, KT_TILES, N], bf16, kind="ExternalInput")
    ctxd = nc.dram_tensor("ctxd", [P, KT_TILES, M], bf16, kind="ExternalInput")
    wq = nc.dram_tensor("wq", [P, KT_TILES, IC], bf16, kind="ExternalInput")
    wk = nc.dram_tensor("wk", [P, KT_TILES, IC], bf16, kind="ExternalInput")
    wv = nc.dram_tensor("wv", [P, KT_TILES, IC], bf16, kind="ExternalInput")
    wo = nc.dram_tensor("wo", [P, 2, C], bf16, kind="ExternalInput")
    out = nc.dram_tensor("out", [N, C], bf16, kind="ExternalOutput")
    with tile.TileContext(nc) as tc:
        for _ in range(reps):
            with ExitStack() as ctx:
                _body(nc, tc, ctx, xd, ctxd, wq, wk, wv, wo, out, opt=opt)
    nc.compile()
    _CACHE[key] = nc
    return nc


def _to_tiled(a, inner):
    """[K*128, inner] f32 -> [128, K, inner] bf16 (partition-major tiling)."""
    k = a.shape[0] // P
    return np.ascontiguousarray(
        a.reshape(k, P, inner).transpose(1, 0, 2).astype(ml_dtypes.bfloat16)
    )


def _shard_inputs(x, context, Wq, Wk, Wv, Wo):
    in_maps = []
    for c in range(NCORES):
        b, g = divmod(c, NCORES // B)
        cols = slice(g * IC, (g + 1) * IC)
        in_maps.append({
            "xd": _to_tiled(np.ascontiguousarray(x[b].T), N),
            "ctxd": _to_tiled(np.ascontiguousarray(context[b].T), M),
            "wq": _to_tiled(np.ascontiguousarray(Wq[:, cols]), IC),
            "wk": _to_tiled(np.ascontiguousarray(Wk[:, cols]), IC),
            "wv": _to_tiled(np.ascontiguousarray(Wv[:, cols]), IC),
            "wo": _to_tiled(np.ascontiguousarray(Wo[cols, :]), C),
        })
    return in_maps


def kernel(x, context, Wq, Wk, Wv, Wo, reps=1):
    x = np.asarray(x, dtype=np.float32)
    context = np.asarray(context, dtype=np.float32)
    Wq, Wk, Wv, Wo = (np.asarray(w, dtype=np.float32) for w in (Wq, Wk, Wv, Wo))
    nc = _build(reps)
    in_maps = _shard_inputs(x, context, Wq, Wk, Wv, Wo)
    res = run_bass_kernel_spmd(nc, in_maps, core_ids=list(range(NCORES)))
    gpb = NCORES // B
    out = np.zeros((B, N, C), dtype=np.float32)
    for c in range(NCORES):
        out[c // gpb] += np.asarray(res.results[c]["out"], dtype=np.float32)
    return out



# revision 2
# speedup vs baseline: 1.0441x; 1.0441x over previous
"""Trainium2 Bass kernel for CrossAttention (B=2, N=M=2048, 16 heads x 64).

Sharding: batch x head-group parallel over 8 cores. Core c handles batch
c//4 and heads [4*(c%4), 4*(c%4)+4). Projection weights are column-split
(Wq/Wk/Wv) / row-split (Wo) per core; each core produces a partial
[2048, 1024] output (bf16) which the host sums per batch (4 partials).

V3 design (cost-model driven, all bf16):
  - Matmul cost = out_free_rows x cycles; contraction dim and output
    partition count are free.  The attn@V matmul therefore runs in
    "layout B": out[n 128, d 65] with lhsT = es[m, n-slice], rhs =
    v[m, 65] -- 66560 rows instead of 131072 (layout A).  Total PE:
    QKV proj 98304 + S 131072 + O 66560 + transpose 4096 + out-proj
    32768 = 332800 rows (138.7us floor at 2.4GHz).
  - es (exp of logits) persists in SBUF bf16 for 2.5 generations
    ((qc, head-pair) chunks); O accumulation is nt-major: each
    (n-tile, head) PSUM accumulator runs its 16 m-chunk matmuls
    back-to-back, so only 2 o-banks are live (PSUM: s 4 + fill-unit 1
    + fill-group 1 + o 2 = 8 banks).
  - Normalization: ones column in V gives denominators in o col 64;
    DVE reciprocal [P,1] + per-partition tensor_scalar_mul; PE
    transpose (identity matmul) flips [n, ic] -> [ic, n] for the
    output projection.
  - ACT runs only the softmax Exp (128 x [128,1024] = 132.9us).
  - Emission scheduling: a priority-class work queue (KT/QT/V/O/FIN
    units) pumped per exp-slot with a cycle budget plus forced drains
    at dependency barriers keeps PE saturated.
"""

import numpy as np
import ml_dtypes
from collections import deque
from contextlib import ExitStack

import concourse.tile as tile
from concourse import bacc, mybir
from concourse.bass_utils import run_bass_kernel_spmd

B, N, M, C = 2, 2048, 2048, 1024
HEADS, D = 16, 64
HPC = 4            # heads per core
IC = HPC * D       # 256 inner dims per core
SCALE = D ** -0.5
NCORES = 8
KT_TILES = C // 128   # 8 contraction tiles for projections
P = 128
MT = M // P           # 16 m tiles
QC = 512
NQC = N // QC         # 4 q chunks
NGEN = 2 * NQC        # 8 (qc, head-pair) generations
f32 = mybir.dt.float32
bf16 = mybir.dt.bfloat16

_CACHE = {}


def _body(nc, tc, ctx, xd, ctxd, wq, wk, wv, wo, identd, out, opt=None):
    opt = opt or {}
    SLOT_BUDGET = opt.get("slot_budget", 2500)
    HEAD_JUNK = opt.get("head_junk", 6)
    MID_JUNK = opt.get("mid_junk", 2)
    ES_BUFS = opt.get("es_bufs", 40)

    const = ctx.enter_context(tc.tile_pool(name="const", bufs=1))
    wq_sb = const.tile([P, KT_TILES, IC], bf16, tag="wq")
    wk_sb = const.tile([P, KT_TILES, IC], bf16, tag="wk")
    wv_sb = const.tile([P, KT_TILES, IC], bf16, tag="wv")
    wo_sb = const.tile([P, 2, C], bf16, tag="wo")
    ctx_sb = const.tile([P, KT_TILES, M], bf16, tag="ctx")
    x_sb = const.tile([P, KT_TILES, N], bf16, tag="x")
    kt_sb = [const.tile([P, M], bf16, tag=f"kt{j}", name=f"kt{j}") for j in range(2)]
    qt_sb = [const.tile([P, N], bf16, tag=f"qt{j}", name=f"qt{j}") for j in range(2)]
    v_sb = const.tile([P, MT, HPC, D + 1], bf16, tag="v")
    ao_sb = [const.tile([P, N], bf16, tag=f"ao{j}", name=f"ao{j}") for j in range(2)]
    ident_sb = const.tile([P, P], bf16, tag="ident")
    ones_sb = const.tile([P, 1], f32, tag="ones")
    junk_sb = const.tile([P, QC], bf16, tag="junk")

    es_pool = ctx.enter_context(tc.tile_pool(name="es", bufs=ES_BUFS))
    norm_pool = ctx.enter_context(tc.tile_pool(name="norm", bufs=2))
    r_pool = ctx.enter_context(tc.tile_pool(name="rp", bufs=4))
    outst_pool = ctx.enter_context(tc.tile_pool(name="outst", bufs=4))

    spool = ctx.enter_context(tc.tile_pool(name="s_ps", bufs=2, space="PSUM"))
    fu = ctx.enter_context(tc.tile_pool(name="fu_ps", bufs=1, space="PSUM"))
    fg = ctx.enter_context(tc.tile_pool(name="fg_ps", bufs=1, space="PSUM"))
    opool = ctx.enter_context(tc.tile_pool(name="o_ps", bufs=2, space="PSUM"))

    # junk tile first on DVE so PE warmup matmuls can start ~0.4us in
    nc.vector.memset(junk_sb[:], 0.0)
    nc.vector.memset(ones_sb[:], 1.0)
    # ones column of V_aug (denominator trick)
    nc.vector.tensor_copy(
        v_sb[:, :, :, D:D + 1],
        ones_sb[:, 0:1].to_broadcast((P, MT, HPC, 1)),
    )

    # ---- input DMAs ----
    # scalar: wk, wq, ident, wv, wo, x qc1-3; sync/gpsimd: ctx (first
    # quarter k-split so KT m0-3 can start early), x qc0.
    for k in range(KT_TILES):
        nc.scalar.dma_start(wk_sb[:, k, :], wk[:, k, :])
    nc.scalar.dma_start(wq_sb[:], wq[:])
    nc.scalar.dma_start(ident_sb[:], identd[:])
    nc.scalar.dma_start(wv_sb[:], wv[:])
    nc.scalar.dma_start(wo_sb[:], wo[:])
    for k in range(KT_TILES):
        nc.scalar.dma_start(x_sb[:, k, QC:N], xd[:, k, QC:N])
    for k in range(KT_TILES):
        eng = nc.sync if k < 4 else nc.gpsimd
        eng.dma_start(ctx_sb[:, k, 0:QC], ctxd[:, k, 0:QC])
    for k in range(KT_TILES):
        eng = nc.sync if k < 4 else nc.gpsimd
        eng.dma_start(x_sb[:, k, 0:QC], xd[:, k, 0:QC])
    for k in range(KT_TILES):
        eng = nc.sync if k % 2 == 0 else nc.gpsimd
        eng.dma_start(ctx_sb[:, k, QC:M], ctxd[:, k, QC:M])

    def emit_junk(n, tag_n):
        # PE keep-warm matmuls (nothing reads the result): bridge idle
        # windows so the pstate ramp does not reset.
        for i in range(n):
            jp = fu.tile([P, QC], f32, tag="f", name=f"junk_{tag_n}_{i}")
            nc.tensor.matmul(
                jp[:], junk_sb[0:P, 0:P], junk_sb[:, 0:QC],
                start=True, stop=True,
            )

    # ---- fill unit emitters ----
    def kt_unit(p2, mt2, pool, tg):
        kp = pool.tile([P, QC], f32, tag=tg, name=f"kp{p2}_{mt2}")
        for k in range(KT_TILES):
            nc.tensor.matmul(
                kp[:, 0:P],
                wk_sb[:, k, p2 * P:(p2 + 1) * P],
                ctx_sb[:, k, mt2 * P:(mt2 + 1) * P],
                start=(k == 0), stop=(k == KT_TILES - 1),
            )
        nc.vector.tensor_copy(kt_sb[p2][:, mt2 * P:(mt2 + 1) * P], kp[:, 0:P])

    def v_unit(half, mt2, pool, tg):
        vt = pool.tile([P, QC], f32, tag=tg, name=f"vt{half}_{mt2}")
        for k in range(KT_TILES):
            nc.tensor.matmul(
                vt[:, 0:P],
                ctx_sb[:, k, mt2 * P:(mt2 + 1) * P],
                wv_sb[:, k, half * P:(half + 1) * P],
                start=(k == 0), stop=(k == KT_TILES - 1),
            )
        nc.vector.tensor_copy(
            v_sb[:, mt2, 2 * half:2 * half + 2, 0:D],
            vt[:, 0:P].rearrange("p (h d) -> p h d", d=D),
        )

    qt_state = {}

    def qt_item(qc2, j, k):
        if k == 0:
            qt_state[(qc2, j)] = fg.tile([P, QC], f32, tag="g",
                                         name=f"qg{qc2}_{j}")
        t = qt_state[(qc2, j)]
        nc.tensor.matmul(
            t[:],
            wq_sb[:, k, j * P:(j + 1) * P],
            x_sb[:, k, qc2 * QC:(qc2 + 1) * QC],
            start=(k == 0), stop=(k == KT_TILES - 1),
        )
        if k == KT_TILES - 1:
            nc.vector.tensor_copy(qt_sb[j][:, qc2 * QC:(qc2 + 1) * QC], t[:])
            del qt_state[(qc2, j)]

    def fin_unit(nt_g, ec):
        ft = fu.tile([P, QC], f32, tag="f", name=f"fin{nt_g}_{ec}")
        for j in range(2):
            nc.tensor.matmul(
                ft[:],
                ao_sb[j][:, nt_g * P:(nt_g + 1) * P],
                wo_sb[:, j, ec * QC:(ec + 1) * QC],
                start=(j == 0), stop=(j == 1),
            )
        ost = outst_pool.tile([P, QC], bf16, tag="ostg", name=f"og{nt_g}_{ec}")
        nc.vector.tensor_copy(ost[:], ft[:])
        deng = nc.sync if (nt_g + ec) % 2 == 0 else nc.gpsimd
        deng.dma_start(out[nt_g * P:(nt_g + 1) * P, ec * QC:(ec + 1) * QC],
                       ost[:])

    # ---- attention O-chunk machinery (layout B, nt-major) ----
    es_tiles = {}
    o_state = {}
    norm_state = {}

    def o_drain(g2, c):
        qc2, p2 = divmod(g2, 2)
        nt_l, hh = divmod(c, 2)
        nt_g = qc2 * 4 + nt_l
        ot = o_state.pop((g2, c))
        if hh == 0:
            norm_state[(g2, nt_l)] = norm_pool.tile(
                [P, P], bf16, tag="nm", name=f"nm{g2}_{nt_l}")
        nm = norm_state[(g2, nt_l)]
        r = r_pool.tile([P, 1], f32, tag="r", name=f"r{g2}_{c}")
        nc.vector.reciprocal(r[:], ot[:, D:D + 1])
        nc.vector.tensor_scalar_mul(nm[:, hh * D:(hh + 1) * D], ot[:, 0:D], r[:])
        if hh == 1:
            tp = fu.tile([P, P], bf16, tag="f", name=f"tp{g2}_{nt_l}")
            nc.tensor.transpose(tp[:], nm[:], ident_sb[:])
            nc.vector.tensor_copy(ao_sb[p2][:, nt_g * P:(nt_g + 1) * P], tp[:])
            del norm_state[(g2, nt_l)]

    def o_chunk(g2, c):
        qc2, p2 = divmod(g2, 2)
        nt_l, hh = divmod(c, 2)
        h = 2 * p2 + hh
        ot = opool.tile([P, D + 1], f32, tag="o", name=f"o{g2}_{c}")
        for mt2 in range(MT):
            nc.tensor.matmul(
                ot[:, 0:D + 1],
                es_tiles[(g2, mt2)][:, hh * QC + nt_l * P:
                                    hh * QC + (nt_l + 1) * P],
                v_sb[:, mt2, h, :],
                start=(mt2 == 0), stop=(mt2 == MT - 1),
            )
        o_state[(g2, c)] = ot
        if c >= 1:
            o_drain(g2, c - 1)

    # ---- priority-class work queue ----
    # item: (cls, idx, cyc, min_gen, fn)
    queue = deque()

    def seed():
        def add(cls, idx, cyc, min_gen, fn):
            queue.append((cls, idx, cyc, min_gen, fn))

        for i, mt2 in enumerate(range(3, MT)):            # 0: KT j0 m3..15
            add(0, mt2, 1024,  0, (lambda m=mt2, i2=i: kt_unit(
                0, m, opool if i2 % 2 == 0 else fu,
                "o" if i2 % 2 == 0 else "f")))
        for k in range(KT_TILES):                          # 1: QT(qc0, j1)
            add(1, k, QC, 0, (lambda k2=k: qt_item(0, 1, k2)))
        for i, mt2 in enumerate(range(MT)):                # 2: KT j1
            add(2, mt2, 1024, 0, (lambda m=mt2, i2=i: kt_unit(
                1, m, opool if i2 % 2 == 0 else fu,
                "o" if i2 % 2 == 0 else "f")))
        for i, mt2 in enumerate(range(MT)):                # 3: V h01
            add(3, mt2, 1024, 0, (lambda m=mt2, i2=i: v_unit(
                0, m, opool if i2 % 2 == 0 else fu,
                "o" if i2 % 2 == 0 else "f")))

        def add_o(cls, g2):
            for c in range(8):
                add(cls, c, 1100, g2 + 1, (lambda g3=g2, c2=c: o_chunk(g3, c2)))
            add(cls, 8, 150, g2 + 1, (lambda g3=g2: o_drain(g3, 7)))

        def add_qt(cls, qc2, j):
            for k in range(KT_TILES):
                add(cls, k, QC, 0, (lambda q=qc2, j2=j, k2=k: qt_item(q, j2, k2)))

        def add_fin(cls, qc2):
            for nt_l in range(4):
                for ec in range(2):
                    add(cls, nt_l * 2 + ec, 1024, 0,
                        (lambda n=qc2 * 4 + nt_l, e=ec: fin_unit(n, e)))

        add_o(4, 0)
        add_qt(5, 1, 0)
        add_qt(6, 1, 1)
        for mt2 in range(MT):                              # 7: V h23
            add(7, mt2, 1024, 0, (lambda m=mt2: v_unit(1, m, fu, "f")))
        add_o(8, 1)
        add_qt(9, 2, 0)
        add_qt(10, 2, 1)
        add_o(11, 2)
        add_fin(12, 0)
        add_o(13, 3)
        add_qt(14, 3, 0)
        add_qt(15, 3, 1)
        add_fin(16, 1)
        add_o(17, 4)
        add_o(18, 5)
        add_fin(19, 2)
        add_o(20, 6)

    seed()
    QT_CLS = {(0, 1): 1, (1, 0): 5, (1, 1): 6, (2, 0): 9, (2, 1): 10,
              (3, 0): 14, (3, 1): 15}
    O_CLS = {0: 4, 1: 8, 2: 11, 3: 13, 4: 17, 5: 18, 6: 20}
    cur_gen = [0]

    def drain_thru(cls_id):
        spent = 0
        while queue and queue[0][0] <= cls_id:
            cls, idx, cyc, mg, fn = queue.popleft()
            assert mg <= cur_gen[0], f"forced drain of blocked item {cls}/{idx}"
            fn()
            spent += cyc
        return spent

    def drain_units(cls_id, max_idx):
        spent = 0
        while queue and (queue[0][0] < cls_id
                         or (queue[0][0] == cls_id and queue[0][1] <= max_idx)):
            cls, idx, cyc, mg, fn = queue.popleft()
            assert mg <= cur_gen[0]
            fn()
            spent += cyc
        return spent

    def pump(budget):
        spent = 0
        while queue and spent < budget and queue[0][3] <= cur_gen[0]:
            cls, idx, cyc, mg, fn = queue.popleft()
            fn()
            spent += cyc
        return spent

    # ---- head: warmup + KT m0-2 + QT(qc0, j0) via s-pool tiles ----
    emit_junk(HEAD_JUNK, "h")
    h0 = spool.tile([P, 2 * QC], f32, tag="s", name="h0")
    for mloc, off in ((0, 0), (1, QC)):
        for k in range(KT_TILES):
            nc.tensor.matmul(
                h0[:, off:off + P],
                wk_sb[:, k, 0:P],
                ctx_sb[:, k, mloc * P:(mloc + 1) * P],
                start=(k == 0), stop=(k == KT_TILES - 1),
            )
    nc.vector.tensor_copy(kt_sb[0][:, 0:P], h0[:, 0:P])
    nc.vector.tensor_copy(kt_sb[0][:, P:2 * P], h0[:, QC:QC + P])
    h1 = spool.tile([P, 2 * QC], f32, tag="s", name="h1")
    for k in range(KT_TILES):
        nc.tensor.matmul(
            h1[:, QC:QC + P],
            wk_sb[:, k, 0:P],
            ctx_sb[:, k, 2 * P:3 * P],
            start=(k == 0), stop=(k == KT_TILES - 1),
        )
    emit_junk(MID_JUNK, "m")
    for k in range(KT_TILES):
        nc.tensor.matmul(
            h1[:, 0:QC],
            wq_sb[:, k, 0:P],
            x_sb[:, k, 0:QC],
            start=(k == 0), stop=(k == KT_TILES - 1),
        )
    nc.vector.tensor_copy(kt_sb[0][:, 2 * P:3 * P], h1[:, QC:QC + P])
    nc.vector.tensor_copy(qt_sb[0][:, 0:QC], h1[:, 0:QC])

    # ---- main loop: 8 generations x 16 exp slots ----
    for g in range(NGEN):
        cur_gen[0] = g
        qc, p = divmod(g, 2)
        for mt in range(MT):
            spent = 0
            if g == 0:
                spent += drain_units(0, mt)
            elif g == 1:
                spent += drain_units(2, mt)
            elif mt == 0:
                spent += drain_thru(QT_CLS[(qc, p)])
            elif mt == 8:
                spent += drain_thru(O_CLS[g - 2])
            s_t = spool.tile([P, 2 * QC], f32, tag="s", name=f"s{g}_{mt}")
            for hh in range(2):
                nc.tensor.matmul(
                    s_t[:, hh * QC:(hh + 1) * QC],
                    kt_sb[p][hh * D:(hh + 1) * D, mt * P:(mt + 1) * P],
                    qt_sb[p][hh * D:(hh + 1) * D, qc * QC:(qc + 1) * QC],
                    start=True, stop=True,
                )
            spent += 2 * QC
            es_t = es_pool.tile([P, 2 * QC], bf16, tag="es", name=f"es{g}_{mt}")
            nc.scalar.activation(
                es_t[:], s_t[:],
                mybir.ActivationFunctionType.Exp, scale=SCALE,
            )
            es_tiles[(g, mt)] = es_t
            pump(SLOT_BUDGET - spent)

    # ---- tail: O(gen 7) + output projection for qc3 ----
    cur_gen[0] = NGEN
    drain_thru(20)
    g7 = NGEN - 1
    for c in range(8):
        o_chunk(g7, c)
        if c >= 2 and c % 2 == 0:
            nt_l = (c - 2) // 2
            fin_unit(12 + nt_l, 0)
            fin_unit(12 + nt_l, 1)
    o_drain(g7, 7)
    fin_unit(15, 0)
    fin_unit(15, 1)


def _build(reps=1, opt=None):
    key = (reps, tuple(sorted((opt or {}).items())))
    if key in _CACHE:
        return _CACHE[key]
    nc = bacc.Bacc("TRN2", target_bir_lowering=False, debug=False)
    xd = nc.dram_tensor("xd", [P, KT_TILES, N], bf16, kind="ExternalInput")
    ctxd = nc.dram_tensor("ctxd", [P, KT_TILES, M], bf16, kind="ExternalInput")
    wq = nc.dram_tensor("wq", [P, KT_TILES, IC], bf16, kind="ExternalInput")
    wk = nc.dram_tensor("wk", [P, KT_TILES, IC], bf16, kind="ExternalInput")
    wv = nc.dram_tensor("wv", [P, KT_TILES, IC], bf16, kind="ExternalInput")
    wo = nc.dram_tensor("wo", [P, 2, C], bf16, kind="ExternalInput")
    identd = nc.dram_tensor("ident", [P, P], bf16, kind="ExternalInput")
    out = nc.dram_tensor("out", [N, C], bf16, kind="ExternalOutput")
    with tile.TileContext(nc) as tc:
        for _ in range(reps):
            with ExitStack() as ctx:
                _body(nc, tc, ctx, xd, ctxd, wq, wk, wv, wo, identd, out,
                      opt=opt)
    nc.compile()
    _CACHE[key] = nc
    return nc


def _to_tiled(a, inner):
    """[K*128, inner] f32 -> [128, K, inner] bf16 (partition-major tiling)."""
    k = a.shape[0] // P
    return np.ascontiguousarray(
        a.reshape(k, P, inner).transpose(1, 0, 2).astype(ml_dtypes.bfloat16)
    )


def _shard_inputs(x, context, Wq, Wk, Wv, Wo):
    ident = np.eye(P, dtype=ml_dtypes.bfloat16)
    in_maps = []
    for c in range(NCORES):
        b, g = divmod(c, NCORES // B)
        cols = slice(g * IC, (g + 1) * IC)
        in_maps.append({
            "xd": _to_tiled(np.ascontiguousarray(x[b].T), N),
            "ctxd": _to_tiled(np.ascontiguousarray(context[b].T), M),
            "wq": _to_tiled(np.ascontiguousarray(Wq[:, cols]), IC),
            "wk": _to_tiled(np.ascontiguousarray(Wk[:, cols]), IC),
            "wv": _to_tiled(np.ascontiguousarray(Wv[:, cols]), IC),
            "wo": _to_tiled(np.ascontiguousarray(Wo[cols, :]), C),
            "ident": ident,
        })
    return in_maps


def kernel(x, context, Wq, Wk, Wv, Wo, reps=1):
    x = np.asarray(x, dtype=np.float32)
    context = np.asarray(context, dtype=np.float32)
    Wq, Wk, Wv, Wo = (np.asarray(w, dtype=np.float32) for w in (Wq, Wk, Wv, Wo))
    nc = _build(reps)
    in_maps = _shard_inputs(x, context, Wq, Wk, Wv, Wo)
    res = run_bass_kernel_spmd(nc, in_maps, core_ids=list(range(NCORES)))
    gpb = NCORES // B
    out = np.zeros((B, N, C), dtype=np.float32)
    for c in range(NCORES):
        out[c // gpb] += np.asarray(res.results[c]["out"], dtype=np.float32)
    return out


# revision 5
# speedup vs baseline: 1.0460x; 1.0019x over previous
"""Trainium2 Bass kernel for CrossAttention (B=2, N=M=2048, 16 heads x 64).

Sharding: batch x head-group parallel over 8 cores. Core c handles batch
c//4 and heads [4*(c%4), 4*(c%4)+4). Projection weights are column-split
(Wq/Wk/Wv) / row-split (Wo) per core; each core produces a partial
[2048, 1024] output (bf16) which the host sums per batch (4 partials).

V3 design (cost-model driven, all bf16):
  - Matmul cost = out_free_rows x cycles; contraction dim and output
    partition count are free.  The attn@V matmul therefore runs in
    "layout B": out[n 128, d 65] with lhsT = es[m, n-slice], rhs =
    v[m, 65] -- 66560 rows instead of 131072 (layout A).  Total PE:
    QKV proj 98304 + S 131072 + O 66560 + transpose 4096 + out-proj
    32768 = 332800 rows (138.7us floor at 2.4GHz).
  - es (exp of logits) persists in SBUF bf16 for 2.5 generations
    ((qc, head-pair) chunks); O accumulation is nt-major: each
    (n-tile, head) PSUM accumulator runs its 16 m-chunk matmuls
    back-to-back, so only 2 o-banks are live (PSUM: s 4 + fill-unit 1
    + fill-group 1 + o 2 = 8 banks).
  - Normalization: ones column in V gives denominators in o col 64;
    DVE reciprocal [P,1] + per-partition tensor_scalar_mul; PE
    transpose (identity matmul) flips [n, ic] -> [ic, n] for the
    output projection.
  - ACT runs only the softmax Exp (128 x [128,1024] = 132.9us).
  - Emission scheduling: a priority-class work queue (KT/QT/V/O/FIN
    units) pumped per exp-slot with a cycle budget plus forced drains
    at dependency barriers keeps PE saturated.
"""

import numpy as np
import ml_dtypes
from collections import deque
from contextlib import ExitStack

import concourse.tile as tile
from concourse import bacc, mybir
from concourse.bass_utils import run_bass_kernel_spmd

B, N, M, C = 2, 2048, 2048, 1024
HEADS, D = 16, 64
HPC = 4            # heads per core
IC = HPC * D       # 256 inner dims per core
SCALE = D ** -0.5
NCORES = 8
KT_TILES = C // 128   # 8 contraction tiles for projections
P = 128
MT = M // P           # 16 m tiles
QC = 512
NQC = N // QC         # 4 q chunks
NGEN = 2 * NQC        # 8 (qc, head-pair) generations
f32 = mybir.dt.float32
bf16 = mybir.dt.bfloat16

_CACHE = {}


def _body(nc, tc, ctx, xd, ctxd, wq, wk, wv, wo, identd, out, opt=None):
    opt = opt or {}
    SLOT_BUDGET = opt.get("slot_budget", 2500)
    HEAD_JUNK = opt.get("head_junk", 6)
    MID_JUNK = opt.get("mid_junk", 2)
    ES_BUFS = opt.get("es_bufs", 40)

    const = ctx.enter_context(tc.tile_pool(name="const", bufs=1))
    wq_sb = const.tile([P, KT_TILES, IC], bf16, tag="wq")
    wk_sb = const.tile([P, KT_TILES, IC], bf16, tag="wk")
    wv_sb = const.tile([P, KT_TILES, IC], bf16, tag="wv")
    wo_sb = const.tile([P, 2, C], bf16, tag="wo")
    ctx_sb = const.tile([P, KT_TILES, M], bf16, tag="ctx")
    x_sb = const.tile([P, KT_TILES, N], bf16, tag="x")
    kt_sb = [const.tile([P, M], bf16, tag=f"kt{j}", name=f"kt{j}") for j in range(2)]
    qt_sb = [const.tile([P, N], bf16, tag=f"qt{j}", name=f"qt{j}") for j in range(2)]
    v_sb = const.tile([P, MT, HPC, D + 1], bf16, tag="v")
    ao_sb = [const.tile([P, N], bf16, tag=f"ao{j}", name=f"ao{j}") for j in range(2)]
    ident_sb = const.tile([P, P], bf16, tag="ident")
    ones_sb = const.tile([P, 1], f32, tag="ones")
    junk_sb = const.tile([P, QC], bf16, tag="junk")

    es_pool = ctx.enter_context(tc.tile_pool(name="es", bufs=ES_BUFS))
    norm_pool = ctx.enter_context(tc.tile_pool(name="norm", bufs=2))
    r_pool = ctx.enter_context(tc.tile_pool(name="rp", bufs=4))
    outst_pool = ctx.enter_context(tc.tile_pool(name="outst", bufs=4))

    spool = ctx.enter_context(tc.tile_pool(name="s_ps", bufs=2, space="PSUM"))
    fu = ctx.enter_context(tc.tile_pool(name="fu_ps", bufs=1, space="PSUM"))
    fg = ctx.enter_context(tc.tile_pool(name="fg_ps", bufs=1, space="PSUM"))
    opool = ctx.enter_context(tc.tile_pool(name="o_ps", bufs=2, space="PSUM"))

    # junk tile first on DVE so PE warmup matmuls can start ~0.4us in
    nc.vector.memset(junk_sb[:], 0.0)
    nc.vector.memset(ones_sb[:], 1.0)
    # ones column of V_aug (denominator trick)
    nc.vector.tensor_copy(
        v_sb[:, :, :, D:D + 1],
        ones_sb[:, 0:1].to_broadcast((P, MT, HPC, 1)),
    )

    # ---- input DMAs ----
    # ACT must stay nearly DMA-free (exp starts ~8us and dma_start
    # occupies the issuing engine queue for the whole transfer): scalar
    # gets only the small early weights; everything else on SP/Pool.
    nc.scalar.dma_start(wk_sb[:], wk[:])
    nc.scalar.dma_start(wq_sb[:], wq[:])
    nc.scalar.dma_start(ident_sb[:], identd[:])
    for k in range(KT_TILES):
        eng = nc.sync if k < 4 else nc.gpsimd
        eng.dma_start(ctx_sb[:, k, 0:QC], ctxd[:, k, 0:QC])
    for k in range(KT_TILES):
        eng = nc.sync if k < 4 else nc.gpsimd
        eng.dma_start(x_sb[:, k, 0:QC], xd[:, k, 0:QC])
    for k in range(KT_TILES):
        eng = nc.sync if k % 2 == 0 else nc.gpsimd
        eng.dma_start(ctx_sb[:, k, QC:M], ctxd[:, k, QC:M])
    nc.sync.dma_start(wv_sb[:], wv[:])
    nc.gpsimd.dma_start(wo_sb[:], wo[:])
    for k in range(KT_TILES):
        eng = nc.sync if k % 2 == 0 else nc.gpsimd
        eng.dma_start(x_sb[:, k, QC:N], xd[:, k, QC:N])

    def emit_junk(n, tag_n):
        # PE keep-warm matmuls (nothing reads the result): bridge idle
        # windows so the pstate ramp does not reset.
        for i in range(n):
            jp = fu.tile([P, QC], f32, tag="f", name=f"junk_{tag_n}_{i}")
            nc.tensor.matmul(
                jp[:], junk_sb[0:P, 0:P], junk_sb[:, 0:QC],
                start=True, stop=True,
            )

    # ---- fill unit emitters ----
    def kt_unit(p2, mt2, pool, tg):
        kp = pool.tile([P, QC], f32, tag=tg, name=f"kp{p2}_{mt2}")
        for k in range(KT_TILES):
            nc.tensor.matmul(
                kp[:, 0:P],
                wk_sb[:, k, p2 * P:(p2 + 1) * P],
                ctx_sb[:, k, mt2 * P:(mt2 + 1) * P],
                start=(k == 0), stop=(k == KT_TILES - 1),
            )
        nc.vector.tensor_copy(kt_sb[p2][:, mt2 * P:(mt2 + 1) * P], kp[:, 0:P])

    def v_unit(half, mt2, pool, tg):
        vt = pool.tile([P, QC], f32, tag=tg, name=f"vt{half}_{mt2}")
        for k in range(KT_TILES):
            nc.tensor.matmul(
                vt[:, 0:P],
                ctx_sb[:, k, mt2 * P:(mt2 + 1) * P],
                wv_sb[:, k, half * P:(half + 1) * P],
                start=(k == 0), stop=(k == KT_TILES - 1),
            )
        nc.vector.tensor_copy(
            v_sb[:, mt2, 2 * half:2 * half + 2, 0:D],
            vt[:, 0:P].rearrange("p (h d) -> p h d", d=D),
        )

    qt_state = {}

    def qt_item(qc2, j, k):
        if k == 0:
            qt_state[(qc2, j)] = fg.tile([P, QC], f32, tag="g",
                                         name=f"qg{qc2}_{j}")
        t = qt_state[(qc2, j)]
        nc.tensor.matmul(
            t[:],
            wq_sb[:, k, j * P:(j + 1) * P],
            x_sb[:, k, qc2 * QC:(qc2 + 1) * QC],
            start=(k == 0), stop=(k == KT_TILES - 1),
        )
        if k == KT_TILES - 1:
            nc.vector.tensor_copy(qt_sb[j][:, qc2 * QC:(qc2 + 1) * QC], t[:])
            del qt_state[(qc2, j)]

    def fin_unit(nt_g, ec, stage_act=False):
        ft = fu.tile([P, QC], f32, tag="f", name=f"fin{nt_g}_{ec}")
        for j in range(2):
            nc.tensor.matmul(
                ft[:],
                ao_sb[j][:, nt_g * P:(nt_g + 1) * P],
                wo_sb[:, j, ec * QC:(ec + 1) * QC],
                start=(j == 0), stop=(j == 1),
            )
        ost = outst_pool.tile([P, QC], bf16, tag="ostg", name=f"og{nt_g}_{ec}")
        if stage_act:
            nc.scalar.copy(ost[:], ft[:])
        else:
            nc.vector.tensor_copy(ost[:], ft[:])
        deng = nc.sync if (nt_g + ec) % 2 == 0 else nc.gpsimd
        deng.dma_start(out[nt_g * P:(nt_g + 1) * P, ec * QC:(ec + 1) * QC],
                       ost[:])

    # ---- attention O-chunk machinery (layout B, nt-major) ----
    es_tiles = {}
    o_state = {}
    norm_state = {}

    def o_drain(g2, c):
        qc2, p2 = divmod(g2, 2)
        nt_l, hh = divmod(c, 2)
        nt_g = qc2 * 4 + nt_l
        ot = o_state.pop((g2, c))
        if hh == 0:
            norm_state[(g2, nt_l)] = norm_pool.tile(
                [P, P], bf16, tag="nm", name=f"nm{g2}_{nt_l}")
        nm = norm_state[(g2, nt_l)]
        r = r_pool.tile([P, 1], f32, tag="r", name=f"r{g2}_{c}")
        nc.vector.reciprocal(r[:], ot[:, D:D + 1])
        nc.vector.tensor_scalar_mul(nm[:, hh * D:(hh + 1) * D], ot[:, 0:D], r[:])
        if hh == 1:
            tp = fu.tile([P, P], bf16, tag="f", name=f"tp{g2}_{nt_l}")
            nc.tensor.transpose(tp[:], nm[:], ident_sb[:])
            nc.vector.tensor_copy(ao_sb[p2][:, nt_g * P:(nt_g + 1) * P], tp[:])
            del norm_state[(g2, nt_l)]

    def o_chunk(g2, c):
        qc2, p2 = divmod(g2, 2)
        nt_l, hh = divmod(c, 2)
        h = 2 * p2 + hh
        ot = opool.tile([P, D + 1], f32, tag="o", name=f"o{g2}_{c}")
        for mt2 in range(MT):
            nc.tensor.matmul(
                ot[:, 0:D + 1],
                es_tiles[(g2, mt2)][:, hh * QC + nt_l * P:
                                    hh * QC + (nt_l + 1) * P],
                v_sb[:, mt2, h, :],
                start=(mt2 == 0), stop=(mt2 == MT - 1),
            )
        o_state[(g2, c)] = ot
        if c >= 1:
            o_drain(g2, c - 1)

    # ---- priority-class work queue ----
    # item: (cls, idx, cyc, min_gen, fn)
    queue = deque()

    def seed():
        def add(cls, idx, cyc, min_gen, fn):
            queue.append((cls, idx, cyc, min_gen, fn))

        for i, mt2 in enumerate(range(3, MT)):            # 0: KT j0 m3..15
            add(0, mt2, 1024,  0, (lambda m=mt2, i2=i: kt_unit(
                0, m, opool if i2 % 2 == 0 else fu,
                "o" if i2 % 2 == 0 else "f")))
        for k in range(KT_TILES):                          # 1: QT(qc0, j1)
            add(1, k, QC, 0, (lambda k2=k: qt_item(0, 1, k2)))
        for i, mt2 in enumerate(range(MT)):                # 2: KT j1
            add(2, mt2, 1024, 0, (lambda m=mt2, i2=i: kt_unit(
                1, m, opool if i2 % 2 == 0 else fu,
                "o" if i2 % 2 == 0 else "f")))
        for i, mt2 in enumerate(range(MT)):                # 3: V h01
            add(3, mt2, 1024, 0, (lambda m=mt2, i2=i: v_unit(
                0, m, opool if i2 % 2 == 0 else fu,
                "o" if i2 % 2 == 0 else "f")))

        def add_o(cls, g2):
            for c in range(8):
                add(cls, c, 1100, g2 + 1, (lambda g3=g2, c2=c: o_chunk(g3, c2)))
            add(cls, 8, 150, g2 + 1, (lambda g3=g2: o_drain(g3, 7)))

        def add_qt(cls, qc2, j):
            for k in range(KT_TILES):
                add(cls, k, QC, 0, (lambda q=qc2, j2=j, k2=k: qt_item(q, j2, k2)))

        def add_fin(cls, qc2):
            for nt_l in range(4):
                for ec in range(2):
                    add(cls, nt_l * 2 + ec, 1024, 0,
                        (lambda n=qc2 * 4 + nt_l, e=ec: fin_unit(n, e)))

        add_o(4, 0)
        add_qt(5, 1, 0)
        add_qt(6, 1, 1)
        for mt2 in range(MT):                              # 7: V h23
            add(7, mt2, 1024, 0, (lambda m=mt2: v_unit(1, m, fu, "f")))
        add_o(8, 1)
        add_qt(9, 2, 0)
        add_qt(10, 2, 1)
        add_o(11, 2)
        add_fin(12, 0)
        add_o(13, 3)
        add_qt(14, 3, 0)
        add_qt(15, 3, 1)
        add_fin(16, 1)
        add_o(17, 4)
        add_o(18, 5)
        add_fin(19, 2)
        add_o(20, 6)

    seed()
    QT_CLS = {(0, 1): 1, (1, 0): 5, (1, 1): 6, (2, 0): 9, (2, 1): 10,
              (3, 0): 14, (3, 1): 15}
    O_CLS = {0: 4, 1: 8, 2: 11, 3: 13, 4: 17, 5: 18, 6: 20}
    cur_gen = [0]

    def drain_thru(cls_id):
        spent = 0
        while queue and queue[0][0] <= cls_id:
            cls, idx, cyc, mg, fn = queue.popleft()
            assert mg <= cur_gen[0], f"forced drain of blocked item {cls}/{idx}"
            fn()
            spent += cyc
        return spent

    def drain_units(cls_id, max_idx):
        spent = 0
        while queue and (queue[0][0] < cls_id
                         or (queue[0][0] == cls_id and queue[0][1] <= max_idx)):
            cls, idx, cyc, mg, fn = queue.popleft()
            assert mg <= cur_gen[0]
            fn()
            spent += cyc
        return spent

    def pump(budget):
        spent = 0
        while queue and spent < budget and queue[0][3] <= cur_gen[0]:
            cls, idx, cyc, mg, fn = queue.popleft()
            fn()
            spent += cyc
        return spent

    # ---- head: warmup + KT m0-2 + QT(qc0, j0) via s-pool tiles ----
    emit_junk(HEAD_JUNK, "h")
    h0 = spool.tile([P, 2 * QC], f32, tag="s", name="h0")
    for mloc, off in ((0, 0), (1, QC)):
        for k in range(KT_TILES):
            nc.tensor.matmul(
                h0[:, off:off + P],
                wk_sb[:, k, 0:P],
                ctx_sb[:, k, mloc * P:(mloc + 1) * P],
                start=(k == 0), stop=(k == KT_TILES - 1),
            )
    nc.vector.tensor_copy(kt_sb[0][:, 0:P], h0[:, 0:P])
    nc.vector.tensor_copy(kt_sb[0][:, P:2 * P], h0[:, QC:QC + P])
    h1 = spool.tile([P, 2 * QC], f32, tag="s", name="h1")
    for k in range(KT_TILES):
        nc.tensor.matmul(
            h1[:, QC:QC + P],
            wk_sb[:, k, 0:P],
            ctx_sb[:, k, 2 * P:3 * P],
            start=(k == 0), stop=(k == KT_TILES - 1),
        )
    emit_junk(MID_JUNK, "m")
    for k in range(KT_TILES):
        nc.tensor.matmul(
            h1[:, 0:QC],
            wq_sb[:, k, 0:P],
            x_sb[:, k, 0:QC],
            start=(k == 0), stop=(k == KT_TILES - 1),
        )
    nc.vector.tensor_copy(kt_sb[0][:, 2 * P:3 * P], h1[:, QC:QC + P])
    nc.vector.tensor_copy(qt_sb[0][:, 0:QC], h1[:, 0:QC])

    # ---- main loop: 8 generations x 16 exp slots ----
    for g in range(NGEN):
        cur_gen[0] = g
        qc, p = divmod(g, 2)
        for mt in range(MT):
            spent = 0
            if g == 0:
                spent += drain_units(0, mt)
            elif g == 1:
                spent += drain_units(2, mt)
            elif mt == 0:
                spent += drain_thru(QT_CLS[(qc, p)])
            elif mt == 8:
                spent += drain_thru(O_CLS[g - 2])
            s_t = spool.tile([P, 2 * QC], f32, tag="s", name=f"s{g}_{mt}")
            for hh in range(2):
                nc.tensor.matmul(
                    s_t[:, hh * QC:(hh + 1) * QC],
                    kt_sb[p][hh * D:(hh + 1) * D, mt * P:(mt + 1) * P],
                    qt_sb[p][hh * D:(hh + 1) * D, qc * QC:(qc + 1) * QC],
                    start=True, stop=True,
                )
            spent += 2 * QC
            es_t = es_pool.tile([P, 2 * QC], bf16, tag="es", name=f"es{g}_{mt}")
            nc.scalar.activation(
                es_t[:], s_t[:],
                mybir.ActivationFunctionType.Exp, scale=SCALE,
            )
            es_tiles[(g, mt)] = es_t
            pump(SLOT_BUDGET - spent)

    # ---- tail: O(gen 7) + output projection for qc3 ----
    # All chunks first (PE runs back-to-back), then the fins: keeps
    # DVE round-trips off the in-order PE queue's critical path. Tail
    # stage copies go to ACT (idle after the last exp).
    cur_gen[0] = NGEN
    drain_thru(20)
    g7 = NGEN - 1
    for c in range(8):
        o_chunk(g7, c)
    o_drain(g7, 7)
    for nt_l in range(4):
        fin_unit(12 + nt_l, 0, stage_act=True)
        fin_unit(12 + nt_l, 1, stage_act=True)


def _build(reps=1, opt=None):
    key = (reps, tuple(sorted((opt or {}).items())))
    if key in _CACHE:
        return _CACHE[key]
    nc = bacc.Bacc("TRN2", target_bir_lowering=False, debug=False)
    xd = nc.dram_tensor("xd", [P, KT_TILES, N], bf16, kind="ExternalInput")
    ctxd = nc.dram_tensor("ctxd", [P, KT_TILES, M], bf16, kind="ExternalInput")
    wq = nc.dram_tensor("wq", [P, KT_TILES, IC], bf16, kind="ExternalInput")
    wk = nc.dram_tensor("wk", [P, KT_TILES, IC], bf16, kind="ExternalInput")
    wv = nc.dram_tensor("wv", [P, KT_TILES, IC], bf16, kind="ExternalInput")
    wo = nc.dram_tensor("wo", [P, 2, C], bf16, kind="ExternalInput")
    identd = nc.dram_tensor("ident", [P, P], bf16, kind="ExternalInput")
    out = nc.dram_tensor("out", [N, C], bf16, kind="ExternalOutput")
    with tile.TileContext(nc) as tc:
        for _ in range(reps):
            with ExitStack() as ctx:
                _body(nc, tc, ctx, xd, ctxd, wq, wk, wv, wo, identd, out,
                      opt=opt)
    nc.compile()
    _CACHE[key] = nc
    return nc


def _to_tiled(a, inner):
    """[K*128, inner] f32 -> [128, K, inner] bf16 (partition-major tiling)."""
    k = a.shape[0] // P
    return np.ascontiguousarray(
        a.reshape(k, P, inner).transpose(1, 0, 2).astype(ml_dtypes.bfloat16)
    )


def _shard_inputs(x, context, Wq, Wk, Wv, Wo):
    ident = np.eye(P, dtype=ml_dtypes.bfloat16)
    in_maps = []
    for c in range(NCORES):
        b, g = divmod(c, NCORES // B)
        cols = slice(g * IC, (g + 1) * IC)
        in_maps.append({
            "xd": _to_tiled(np.ascontiguousarray(x[b].T), N),
            "ctxd": _to_tiled(np.ascontiguousarray(context[b].T), M),
            "wq": _to_tiled(np.ascontiguousarray(Wq[:, cols]), IC),
            "wk": _to_tiled(np.ascontiguousarray(Wk[:, cols]), IC),
            "wv": _to_tiled(np.ascontiguousarray(Wv[:, cols]), IC),
            "wo": _to_tiled(np.ascontiguousarray(Wo[cols, :]), C),
            "ident": ident,
        })
    return in_maps


def kernel(x, context, Wq, Wk, Wv, Wo, reps=1):
    x = np.asarray(x, dtype=np.float32)
    context = np.asarray(context, dtype=np.float32)
    Wq, Wk, Wv, Wo = (np.asarray(w, dtype=np.float32) for w in (Wq, Wk, Wv, Wo))
    nc = _build(reps)
    in_maps = _shard_inputs(x, context, Wq, Wk, Wv, Wo)
    res = run_bass_kernel_spmd(nc, in_maps, core_ids=list(range(NCORES)))
    gpb = NCORES // B
    out = np.zeros((B, N, C), dtype=np.float32)
    for c in range(NCORES):
        out[c // gpb] += np.asarray(res.results[c]["out"], dtype=np.float32)
    return out


# revision 8
# speedup vs baseline: 1.0749x; 1.0277x over previous
"""Trainium2 Bass kernel for CrossAttention (B=2, N=M=2048, 16 heads x 64).

Sharding: batch x head-group parallel over 8 cores. Core c handles batch
c//4 and heads [4*(c%4), 4*(c%4)+4). Projection weights are column-split
(Wq/Wk/Wv) / row-split (Wo) per core; each core produces a partial
[2048, 1024] output (bf16) which the host sums per batch (4 partials).

V3 design (cost-model driven, all bf16):
  - Matmul cost = out_free_rows x cycles; contraction dim and output
    partition count are free.  The attn@V matmul therefore runs in
    "layout B": out[n 128, d 65] with lhsT = es[m, n-slice], rhs =
    v[m, 65] -- 66560 rows instead of 131072 (layout A).  Total PE:
    QKV proj 98304 + S 131072 + O 66560 + transpose 4096 + out-proj
    32768 = 332800 rows (138.7us floor at 2.4GHz).
  - es (exp of logits) persists in SBUF bf16 for 2.5 generations
    ((qc, head-pair) chunks); O accumulation is nt-major: each
    (n-tile, head) PSUM accumulator runs its 16 m-chunk matmuls
    back-to-back, so only 2 o-banks are live (PSUM: s 4 + fill-unit 1
    + fill-group 1 + o 2 = 8 banks).
  - Normalization: ones column in V gives denominators in o col 64;
    DVE reciprocal [P,1] + per-partition tensor_scalar_mul; PE
    transpose (identity matmul) flips [n, ic] -> [ic, n] for the
    output projection.
  - ACT runs only the softmax Exp (128 x [128,1024] = 132.9us).
  - Emission scheduling: a priority-class work queue (KT/QT/V/O/FIN
    units) pumped per exp-slot with a cycle budget plus forced drains
    at dependency barriers keeps PE saturated.
"""

import numpy as np
import ml_dtypes
from collections import deque
from contextlib import ExitStack

import concourse.tile as tile
from concourse import bacc, mybir
from concourse.bass_utils import run_bass_kernel_spmd

B, N, M, C = 2, 2048, 2048, 1024
HEADS, D = 16, 64
HPC = 4            # heads per core
IC = HPC * D       # 256 inner dims per core
SCALE = D ** -0.5
NCORES = 8
KT_TILES = C // 128   # 8 contraction tiles for projections
P = 128
MT = M // P           # 16 m tiles
QC = 512
NQC = N // QC         # 4 q chunks
NGEN = 2 * NQC        # 8 (qc, head-pair) generations
f32 = mybir.dt.float32
bf16 = mybir.dt.bfloat16

_CACHE = {}


def _body(nc, tc, ctx, xd, ctxd, wq, wk, wv, wo, identd, out, opt=None):
    opt = opt or {}
    SLOT_BUDGET = opt.get("slot_budget", 2500)
    HEAD_JUNK = opt.get("head_junk", 6)
    MID_JUNK = opt.get("mid_junk", 2)
    ES_BUFS = opt.get("es_bufs", 40)

    const = ctx.enter_context(tc.tile_pool(name="const", bufs=1))
    wq_sb = const.tile([P, KT_TILES, IC], bf16, tag="wq")
    wk_sb = const.tile([P, KT_TILES, IC], bf16, tag="wk")
    wv_sb = const.tile([P, KT_TILES, IC], bf16, tag="wv")
    wo_sb = const.tile([P, 2, C], bf16, tag="wo")
    ctx_sb = const.tile([P, KT_TILES, M], bf16, tag="ctx")
    x_sb = const.tile([P, KT_TILES, N], bf16, tag="x")
    kt_sb = [const.tile([P, M], bf16, tag=f"kt{j}", name=f"kt{j}") for j in range(2)]
    qt_sb = [const.tile([P, N], bf16, tag=f"qt{j}", name=f"qt{j}") for j in range(2)]
    v_sb = const.tile([P, MT, HPC, D + 1], bf16, tag="v")
    ao_sb = [const.tile([P, N], bf16, tag=f"ao{j}", name=f"ao{j}") for j in range(2)]
    ident_sb = const.tile([P, P], bf16, tag="ident")
    ones_sb = const.tile([P, 1], f32, tag="ones")
    junk_sb = const.tile([P, QC], bf16, tag="junk")

    es_pool = ctx.enter_context(tc.tile_pool(name="es", bufs=ES_BUFS))
    norm_pool = ctx.enter_context(tc.tile_pool(name="norm", bufs=2))
    r_pool = ctx.enter_context(tc.tile_pool(name="rp", bufs=4))
    outst_pool = ctx.enter_context(tc.tile_pool(name="outst", bufs=4))

    spool = ctx.enter_context(tc.tile_pool(name="s_ps", bufs=2, space="PSUM"))
    fu = ctx.enter_context(tc.tile_pool(name="fu_ps", bufs=1, space="PSUM"))
    fg = ctx.enter_context(tc.tile_pool(name="fg_ps", bufs=1, space="PSUM"))
    opool = ctx.enter_context(tc.tile_pool(name="o_ps", bufs=2, space="PSUM"))

    # junk tile first on DVE so PE warmup matmuls can start ~0.4us in
    nc.vector.memset(junk_sb[:], 0.0)
    nc.vector.memset(ones_sb[:], 1.0)
    # ones column of V_aug (denominator trick)
    nc.vector.tensor_copy(
        v_sb[:, :, :, D:D + 1],
        ones_sb[:, 0:1].to_broadcast((P, MT, HPC, 1)),
    )

    # ---- input DMAs ----
    # ACT must stay nearly DMA-free (exp starts ~8us and dma_start
    # occupies the issuing engine queue for the whole transfer): scalar
    # gets only the small early weights; everything else on SP/Pool.
    nc.scalar.dma_start(wk_sb[:], wk[:])
    nc.scalar.dma_start(wq_sb[:], wq[:])
    nc.scalar.dma_start(ident_sb[:], identd[:])
    for k in range(KT_TILES):
        eng = nc.sync if k < 4 else nc.gpsimd
        eng.dma_start(ctx_sb[:, k, 0:QC], ctxd[:, k, 0:QC])
    for k in range(KT_TILES):
        eng = nc.sync if k < 4 else nc.gpsimd
        eng.dma_start(x_sb[:, k, 0:QC], xd[:, k, 0:QC])
    for k in range(KT_TILES):
        eng = nc.sync if k % 2 == 0 else nc.gpsimd
        eng.dma_start(ctx_sb[:, k, QC:M], ctxd[:, k, QC:M])
    nc.sync.dma_start(wv_sb[:], wv[:])
    nc.gpsimd.dma_start(wo_sb[:], wo[:])
    for k in range(KT_TILES):
        eng = nc.sync if k % 2 == 0 else nc.gpsimd
        eng.dma_start(x_sb[:, k, QC:N], xd[:, k, QC:N])

    def emit_junk(n, tag_n):
        # PE keep-warm matmuls (nothing reads the result): bridge idle
        # windows so the pstate ramp does not reset.
        for i in range(n):
            jp = fu.tile([P, QC], f32, tag="f", name=f"junk_{tag_n}_{i}")
            nc.tensor.matmul(
                jp[:], junk_sb[0:P, 0:P], junk_sb[:, 0:QC],
                start=True, stop=True,
            )

    # ---- fill unit emitters ----
    def kt_unit(p2, mt2, pool, tg):
        kp = pool.tile([P, QC], f32, tag=tg, name=f"kp{p2}_{mt2}")
        for k in range(KT_TILES):
            nc.tensor.matmul(
                kp[:, 0:P],
                wk_sb[:, k, p2 * P:(p2 + 1) * P],
                ctx_sb[:, k, mt2 * P:(mt2 + 1) * P],
                start=(k == 0), stop=(k == KT_TILES - 1),
            )
        nc.vector.tensor_copy(kt_sb[p2][:, mt2 * P:(mt2 + 1) * P], kp[:, 0:P])

    def v_unit(half, mt2, pool, tg):
        vt = pool.tile([P, QC], f32, tag=tg, name=f"vt{half}_{mt2}")
        for k in range(KT_TILES):
            nc.tensor.matmul(
                vt[:, 0:P],
                ctx_sb[:, k, mt2 * P:(mt2 + 1) * P],
                wv_sb[:, k, half * P:(half + 1) * P],
                start=(k == 0), stop=(k == KT_TILES - 1),
            )
        nc.vector.tensor_copy(
            v_sb[:, mt2, 2 * half:2 * half + 2, 0:D],
            vt[:, 0:P].rearrange("p (h d) -> p h d", d=D),
        )

    qt_state = {}

    def qt_item(qc2, j, k):
        if k == 0:
            qt_state[(qc2, j)] = fg.tile([P, QC], f32, tag="g",
                                         name=f"qg{qc2}_{j}")
        t = qt_state[(qc2, j)]
        nc.tensor.matmul(
            t[:],
            wq_sb[:, k, j * P:(j + 1) * P],
            x_sb[:, k, qc2 * QC:(qc2 + 1) * QC],
            start=(k == 0), stop=(k == KT_TILES - 1),
        )
        if k == KT_TILES - 1:
            nc.vector.tensor_copy(qt_sb[j][:, qc2 * QC:(qc2 + 1) * QC], t[:])
            del qt_state[(qc2, j)]

    def fin_unit(nt_g, ec, ft=None, stage_act=False):
        if ft is None:
            ft = fu.tile([P, QC], f32, tag="f", name=f"fin{nt_g}_{ec}")
        for j in range(2):
            nc.tensor.matmul(
                ft[:],
                ao_sb[j][:, nt_g * P:(nt_g + 1) * P],
                wo_sb[:, j, ec * QC:(ec + 1) * QC],
                start=(j == 0), stop=(j == 1),
            )
        ost = outst_pool.tile([P, QC], bf16, tag="ostg", name=f"og{nt_g}_{ec}")
        if stage_act:
            nc.scalar.copy(ost[:], ft[:])
        else:
            nc.vector.tensor_copy(ost[:], ft[:])
        deng = nc.sync if (nt_g + ec) % 2 == 0 else nc.gpsimd
        deng.dma_start(out[nt_g * P:(nt_g + 1) * P, ec * QC:(ec + 1) * QC],
                       ost[:])

    # ---- attention O-chunk machinery (layout B, nt-major) ----
    es_tiles = {}
    o_state = {}
    norm_state = {}

    def o_drain(g2, c):
        qc2, p2 = divmod(g2, 2)
        nt_l, hh = divmod(c, 2)
        nt_g = qc2 * 4 + nt_l
        ot = o_state.pop((g2, c))
        if hh == 0:
            norm_state[(g2, nt_l)] = norm_pool.tile(
                [P, P], bf16, tag="nm", name=f"nm{g2}_{nt_l}")
        nm = norm_state[(g2, nt_l)]
        r = r_pool.tile([P, 1], f32, tag="r", name=f"r{g2}_{c}")
        nc.vector.reciprocal(r[:], ot[:, D:D + 1])
        nc.vector.tensor_scalar_mul(nm[:, hh * D:(hh + 1) * D], ot[:, 0:D], r[:])
        if hh == 1:
            tp = fu.tile([P, P], bf16, tag="f", name=f"tp{g2}_{nt_l}")
            nc.tensor.transpose(tp[:], nm[:], ident_sb[:])
            nc.vector.tensor_copy(ao_sb[p2][:, nt_g * P:(nt_g + 1) * P], tp[:])
            del norm_state[(g2, nt_l)]

    def o_chunk(g2, c):
        qc2, p2 = divmod(g2, 2)
        nt_l, hh = divmod(c, 2)
        h = 2 * p2 + hh
        ot = opool.tile([P, D + 1], f32, tag="o", name=f"o{g2}_{c}")
        for mt2 in range(MT):
            nc.tensor.matmul(
                ot[:, 0:D + 1],
                es_tiles[(g2, mt2)][:, hh * QC + nt_l * P:
                                    hh * QC + (nt_l + 1) * P],
                v_sb[:, mt2, h, :],
                start=(mt2 == 0), stop=(mt2 == MT - 1),
            )
        o_state[(g2, c)] = ot
        if c >= 1:
            o_drain(g2, c - 1)

    # ---- priority-class work queue ----
    # item: (cls, idx, cyc, min_gen, fn)
    queue = deque()

    def seed():
        def add(cls, idx, cyc, min_gen, fn):
            queue.append((cls, idx, cyc, min_gen, fn))

        for i, mt2 in enumerate(range(3, MT)):            # 0: KT j0 m3..15
            add(0, mt2, 1024,  0, (lambda m=mt2, i2=i: kt_unit(
                0, m, opool if i2 % 2 == 0 else fu,
                "o" if i2 % 2 == 0 else "f")))
        for k in range(KT_TILES):                          # 1: QT(qc0, j1)
            add(1, k, QC, 0, (lambda k2=k: qt_item(0, 1, k2)))
        for i, mt2 in enumerate(range(MT)):                # 2: KT j1
            add(2, mt2, 1024, 0, (lambda m=mt2, i2=i: kt_unit(
                1, m, opool if i2 % 2 == 0 else fu,
                "o" if i2 % 2 == 0 else "f")))
        for i, mt2 in enumerate(range(MT)):                # 3: V h01
            add(3, mt2, 1024, 0, (lambda m=mt2, i2=i: v_unit(
                0, m, opool if i2 % 2 == 0 else fu,
                "o" if i2 % 2 == 0 else "f")))

        def add_o(cls, g2):
            for c in range(8):
                add(cls, c, 1100, g2 + 1, (lambda g3=g2, c2=c: o_chunk(g3, c2)))
            add(cls, 8, 150, g2 + 1, (lambda g3=g2: o_drain(g3, 7)))

        def add_qt(cls, qc2, j):
            for k in range(KT_TILES):
                add(cls, k, QC, 0, (lambda q=qc2, j2=j, k2=k: qt_item(q, j2, k2)))

        def add_fin(cls, qc2):
            for nt_l in range(4):
                for ec in range(2):
                    add(cls, nt_l * 2 + ec, 1024, 0,
                        (lambda n=qc2 * 4 + nt_l, e=ec: fin_unit(n, e)))

        add_o(4, 0)
        add_qt(5, 1, 0)
        add_qt(6, 1, 1)
        for mt2 in range(MT):                              # 7: V h23
            add(7, mt2, 1024, 0, (lambda m=mt2: v_unit(1, m, fu, "f")))
        add_o(8, 1)
        add_qt(9, 2, 0)
        add_qt(10, 2, 1)
        add_o(11, 2)
        add_fin(12, 0)
        add_o(13, 3)
        add_qt(14, 3, 0)
        add_qt(15, 3, 1)
        add_fin(16, 1)
        add_o(17, 4)
        add_o(18, 5)
        add_fin(19, 2)
        add_o(20, 6)

    seed()
    QT_CLS = {(0, 1): 1, (1, 0): 5, (1, 1): 6, (2, 0): 9, (2, 1): 10,
              (3, 0): 14, (3, 1): 15}
    O_CLS = {0: 4, 1: 8, 2: 11, 3: 13, 4: 17, 5: 18, 6: 20}
    cur_gen = [0]

    def drain_thru(cls_id):
        spent = 0
        while queue and queue[0][0] <= cls_id:
            cls, idx, cyc, mg, fn = queue.popleft()
            assert mg <= cur_gen[0], f"forced drain of blocked item {cls}/{idx}"
            fn()
            spent += cyc
        return spent

    def drain_units(cls_id, max_idx):
        spent = 0
        while queue and (queue[0][0] < cls_id
                         or (queue[0][0] == cls_id and queue[0][1] <= max_idx)):
            cls, idx, cyc, mg, fn = queue.popleft()
            assert mg <= cur_gen[0]
            fn()
            spent += cyc
        return spent

    def pump(budget):
        spent = 0
        while queue and spent < budget and queue[0][3] <= cur_gen[0]:
            cls, idx, cyc, mg, fn = queue.popleft()
            fn()
            spent += cyc
        return spent

    # ---- head: warmup + KT m0-2 + QT(qc0, j0) via s-pool tiles ----
    emit_junk(HEAD_JUNK, "h")
    h0 = spool.tile([P, 2 * QC], f32, tag="s", name="h0")
    for mloc, off in ((0, 0), (1, QC)):
        for k in range(KT_TILES):
            nc.tensor.matmul(
                h0[:, off:off + P],
                wk_sb[:, k, 0:P],
                ctx_sb[:, k, mloc * P:(mloc + 1) * P],
                start=(k == 0), stop=(k == KT_TILES - 1),
            )
    nc.vector.tensor_copy(kt_sb[0][:, 0:P], h0[:, 0:P])
    nc.vector.tensor_copy(kt_sb[0][:, P:2 * P], h0[:, QC:QC + P])
    h1 = spool.tile([P, 2 * QC], f32, tag="s", name="h1")
    for k in range(KT_TILES):
        nc.tensor.matmul(
            h1[:, QC:QC + P],
            wk_sb[:, k, 0:P],
            ctx_sb[:, k, 2 * P:3 * P],
            start=(k == 0), stop=(k == KT_TILES - 1),
        )
    emit_junk(MID_JUNK, "m")
    for k in range(KT_TILES):
        nc.tensor.matmul(
            h1[:, 0:QC],
            wq_sb[:, k, 0:P],
            x_sb[:, k, 0:QC],
            start=(k == 0), stop=(k == KT_TILES - 1),
        )
    nc.vector.tensor_copy(kt_sb[0][:, 2 * P:3 * P], h1[:, QC:QC + P])
    nc.vector.tensor_copy(qt_sb[0][:, 0:QC], h1[:, 0:QC])

    # ---- main loop: 8 generations x 16 exp slots ----
    # s matmuls are emitted ONE slot ahead of their exp so the exp
    # stream never eats the s-completion sem latency.
    s_tiles = {}

    def emit_s(g2, mt2):
        qc2, p2 = divmod(g2, 2)
        s_t = spool.tile([P, 2 * QC], f32, tag="s", name=f"s{g2}_{mt2}")
        for hh in range(2):
            nc.tensor.matmul(
                s_t[:, hh * QC:(hh + 1) * QC],
                kt_sb[p2][hh * D:(hh + 1) * D, mt2 * P:(mt2 + 1) * P],
                qt_sb[p2][hh * D:(hh + 1) * D, qc2 * QC:(qc2 + 1) * QC],
                start=True, stop=True,
            )
        s_tiles[(g2, mt2)] = s_t

    emit_s(0, 0)
    for g in range(NGEN):
        cur_gen[0] = g
        qc, p = divmod(g, 2)
        for mt in range(MT):
            spent = 0
            if g == 0:
                spent += drain_units(0, min(mt + 1, MT - 1))
            elif g == 1:
                spent += drain_units(2, min(mt + 1, MT - 1))
            if g >= 2 and mt == 8:
                spent += drain_thru(O_CLS[g - 2])
            es_t = es_pool.tile([P, 2 * QC], bf16, tag="es", name=f"es{g}_{mt}")
            nc.scalar.activation(
                es_t[:], s_tiles.pop((g, mt)),
                mybir.ActivationFunctionType.Exp, scale=SCALE,
            )
            es_tiles[(g, mt)] = es_t
            if mt < MT - 1:
                emit_s(g, mt + 1)
            elif g + 1 < NGEN:
                if g + 1 == 1:
                    spent += drain_units(2, 0)
                elif g + 1 >= 2:
                    spent += drain_thru(QT_CLS[divmod(g + 1, 2)])
                emit_s(g + 1, 0)
            spent += 2 * QC
            pump(SLOT_BUDGET - spent)

    # ---- tail: O(gen 7) + output projection for qc3 ----
    # All chunks first (PE runs back-to-back), then the fins: keeps
    # DVE round-trips off the in-order PE queue's critical path. Tail
    # stage copies go to ACT (idle after the last exp).
    cur_gen[0] = NGEN
    drain_thru(20)
    g7 = NGEN - 1
    for c in range(8):
        o_chunk(g7, c)
    o_drain(g7, 7)
    for nt_l in range(4):
        # tail fins use the (now idle) s-pool banks: 2-deep rotation so
        # consecutive fins don't serialize on a single psum buffer.
        tt = spool.tile([P, 2 * QC], f32, tag="s", name=f"tfin{nt_l}")
        fin_unit(12 + nt_l, 0, ft=tt[:, 0:QC], stage_act=True)
        fin_unit(12 + nt_l, 1, ft=tt[:, QC:2 * QC], stage_act=True)


def _build(reps=1, opt=None):
    key = (reps, tuple(sorted((opt or {}).items())))
    if key in _CACHE:
        return _CACHE[key]
    nc = bacc.Bacc("TRN2", target_bir_lowering=False, debug=False)
    xd = nc.dram_tensor("xd", [P, KT_TILES, N], bf16, kind="ExternalInput")
    ctxd = nc.dram_tensor("ctxd", [P, KT_TILES, M], bf16, kind="ExternalInput")
    wq = nc.dram_tensor("wq", [P, KT_TILES, IC], bf16, kind="ExternalInput")
    wk = nc.dram_tensor("wk", [P, KT_TILES, IC], bf16, kind="ExternalInput")
    wv = nc.dram_tensor("wv", [P, KT_TILES, IC], bf16, kind="ExternalInput")
    wo = nc.dram_tensor("wo", [P, 2, C], bf16, kind="ExternalInput")
    identd = nc.dram_tensor("ident", [P, P], bf16, kind="ExternalInput")
    out = nc.dram_tensor("out", [N, C], bf16, kind="ExternalOutput")
    with tile.TileContext(nc) as tc:
        for _ in range(reps):
            with ExitStack() as ctx:
                _body(nc, tc, ctx, xd, ctxd, wq, wk, wv, wo, identd, out,
                      opt=opt)
    nc.compile()
    _CACHE[key] = nc
    return nc


def _to_tiled(a, inner):
    """[K*128, inner] f32 -> [128, K, inner] bf16 (partition-major tiling)."""
    k = a.shape[0] // P
    return np.ascontiguousarray(
        a.reshape(k, P, inner).transpose(1, 0, 2).astype(ml_dtypes.bfloat16)
    )


def _shard_inputs(x, context, Wq, Wk, Wv, Wo):
    ident = np.eye(P, dtype=ml_dtypes.bfloat16)
    in_maps = []
    for c in range(NCORES):
        b, g = divmod(c, NCORES // B)
        cols = slice(g * IC, (g + 1) * IC)
        in_maps.append({
            "xd": _to_tiled(np.ascontiguousarray(x[b].T), N),
            "ctxd": _to_tiled(np.ascontiguousarray(context[b].T), M),
            "wq": _to_tiled(np.ascontiguousarray(Wq[:, cols]), IC),
            "wk": _to_tiled(np.ascontiguousarray(Wk[:, cols]), IC),
            "wv": _to_tiled(np.ascontiguousarray(Wv[:, cols]), IC),
            "wo": _to_tiled(np.ascontiguousarray(Wo[cols, :]), C),
            "ident": ident,
        })
    return in_maps


def kernel(x, context, Wq, Wk, Wv, Wo, reps=1):
    x = np.asarray(x, dtype=np.float32)
    context = np.asarray(context, dtype=np.float32)
    Wq, Wk, Wv, Wo = (np.asarray(w, dtype=np.float32) for w in (Wq, Wk, Wv, Wo))
    nc = _build(reps)
    in_maps = _shard_inputs(x, context, Wq, Wk, Wv, Wo)
    res = run_bass_kernel_spmd(nc, in_maps, core_ids=list(range(NCORES)))
    gpb = NCORES // B
    out = np.zeros((B, N, C), dtype=np.float32)
    for c in range(NCORES):
        out[c // gpb] += np.asarray(res.results[c]["out"], dtype=np.float32)
    return out


# revision 17
# speedup vs baseline: 1.0827x; 1.0072x over previous
"""Trainium2 Bass kernel for CrossAttention (B=2, N=M=2048, 16 heads x 64).

Sharding: batch x head-group parallel over 8 cores. Core c handles batch
c//4 and heads [4*(c%4), 4*(c%4)+4). Projection weights are column-split
(Wq/Wk/Wv) / row-split (Wo) per core; each core produces a partial
[2048, 1024] output (bf16) which the host sums per batch (4 partials).

V3 design (cost-model driven, all bf16):
  - Matmul cost = out_free_rows x cycles; contraction dim and output
    partition count are free.  The attn@V matmul therefore runs in
    "layout B": out[n 128, d 65] with lhsT = es[m, n-slice], rhs =
    v[m, 65] -- 66560 rows instead of 131072 (layout A).  Total PE:
    QKV proj 98304 + S 131072 + O 66560 + transpose 4096 + out-proj
    32768 = 332800 rows (138.7us floor at 2.4GHz).
  - es (exp of logits) persists in SBUF bf16 for 2.5 generations
    ((qc, head-pair) chunks); O accumulation is nt-major: each
    (n-tile, head) PSUM accumulator runs its 16 m-chunk matmuls
    back-to-back, so only 2 o-banks are live (PSUM: s 4 + fill-unit 1
    + fill-group 1 + o 2 = 8 banks).
  - Normalization: ones column in V gives denominators in o col 64;
    DVE reciprocal [P,1] + per-partition tensor_scalar_mul; PE
    transpose (identity matmul) flips [n, ic] -> [ic, n] for the
    output projection.
  - ACT runs only the softmax Exp (128 x [128,1024] = 132.9us).
  - Emission scheduling: a priority-class work queue (KT/QT/V/O/FIN
    units) pumped per exp-slot with a cycle budget plus forced drains
    at dependency barriers keeps PE saturated.
"""

import numpy as np
import ml_dtypes
from collections import deque
from contextlib import ExitStack

import concourse.tile as tile
from concourse import bacc, mybir
from concourse.bass_utils import run_bass_kernel_spmd

B, N, M, C = 2, 2048, 2048, 1024
HEADS, D = 16, 64
HPC = 4            # heads per core
IC = HPC * D       # 256 inner dims per core
SCALE = D ** -0.5
NCORES = 8
KT_TILES = C // 128   # 8 contraction tiles for projections
P = 128
MT = M // P           # 16 m tiles
QC = 512
NQC = N // QC         # 4 q chunks
NGEN = 2 * NQC        # 8 (qc, head-pair) generations
f32 = mybir.dt.float32
bf16 = mybir.dt.bfloat16

_CACHE = {}


def _body(nc, tc, ctx, xd, ctxd, wq, wk, wv, wo, identd, out, opt=None):
    opt = opt or {}
    SLOT_BUDGET = opt.get("slot_budget", 2500)
    HEAD_JUNK = opt.get("head_junk", 6)
    MID_JUNK = opt.get("mid_junk", 2)
    ES_BUFS = opt.get("es_bufs", 40)

    const = ctx.enter_context(tc.tile_pool(name="const", bufs=1))
    wq_sb = const.tile([P, KT_TILES, IC], bf16, tag="wq")
    wk_sb = const.tile([P, KT_TILES, IC], bf16, tag="wk")
    wv_sb = const.tile([P, KT_TILES, IC], bf16, tag="wv")
    wo_sb = const.tile([P, 2, C], bf16, tag="wo")
    ctx_sb = const.tile([P, KT_TILES, M], bf16, tag="ctx")
    x_sb = const.tile([P, KT_TILES, N], bf16, tag="x")
    kt_sb = [const.tile([P, M], bf16, tag=f"kt{j}", name=f"kt{j}") for j in range(2)]
    qt_sb = [const.tile([P, N], bf16, tag=f"qt{j}", name=f"qt{j}") for j in range(2)]
    v_sb = const.tile([P, MT, HPC, D + 1], bf16, tag="v")
    ao_sb = [const.tile([P, N], bf16, tag=f"ao{j}", name=f"ao{j}") for j in range(2)]
    ident_sb = const.tile([P, P], bf16, tag="ident")
    ones_sb = const.tile([P, 1], f32, tag="ones")
    junk_sb = const.tile([P, QC], bf16, tag="junk")

    es_pool = ctx.enter_context(tc.tile_pool(name="es", bufs=ES_BUFS))
    norm_pool = ctx.enter_context(tc.tile_pool(name="norm", bufs=2))
    r_pool = ctx.enter_context(tc.tile_pool(name="rp", bufs=4))
    outst_pool = ctx.enter_context(tc.tile_pool(name="outst", bufs=4))

    spool = ctx.enter_context(tc.tile_pool(name="s_ps", bufs=2, space="PSUM"))
    fu = ctx.enter_context(tc.tile_pool(name="fu_ps", bufs=1, space="PSUM"))
    fg = ctx.enter_context(tc.tile_pool(name="fg_ps", bufs=1, space="PSUM"))
    opool = ctx.enter_context(tc.tile_pool(name="o_ps", bufs=2, space="PSUM"))

    # junk tile first on DVE so PE warmup matmuls can start ~0.4us in
    nc.vector.memset(junk_sb[:], 0.0)
    nc.vector.memset(ones_sb[:], 1.0)
    # ones column of V_aug (denominator trick)
    nc.vector.tensor_copy(
        v_sb[:, :, :, D:D + 1],
        ones_sb[:, 0:1].to_broadcast((P, MT, HPC, 1)),
    )

    # ---- input DMAs ----
    # ACT must stay nearly DMA-free (exp starts ~8us and dma_start
    # occupies the issuing engine queue for the whole transfer): scalar
    # gets only wk; wq/wv ride the otherwise-idle DVE queue; x qc0
    # goes FIRST on SP/Pool (it gates the head QT -> first exp).
    nc.scalar.dma_start(wq_sb[:], wq[:])
    nc.scalar.dma_start(wk_sb[:], wk[:])
    for k in range(KT_TILES):
        eng = nc.sync if k < 4 else nc.gpsimd
        eng.dma_start(x_sb[:, k, 0:QC], xd[:, k, 0:QC])
    for k in range(KT_TILES):
        eng = nc.sync if k < 4 else nc.gpsimd
        eng.dma_start(ctx_sb[:, k, 0:QC], ctxd[:, k, 0:QC])
    nc.sync.dma_start(wv_sb[:], wv[:])
    for k in range(KT_TILES):
        eng = nc.sync if k % 2 == 0 else nc.gpsimd
        eng.dma_start(ctx_sb[:, k, QC:M], ctxd[:, k, QC:M])
    nc.gpsimd.dma_start(ident_sb[:], identd[:])
    nc.gpsimd.dma_start(wo_sb[:], wo[:])
    for k in range(KT_TILES):
        eng = nc.sync if k % 2 == 0 else nc.gpsimd
        eng.dma_start(x_sb[:, k, QC:N], xd[:, k, QC:N])
    # preload the Exp activation table so exp(0) doesn't pay ATL
    nc.scalar.activation(ones_sb[:], ones_sb[:],
                         mybir.ActivationFunctionType.Exp)

    def emit_junk(n, tag_n):
        # PE keep-warm matmuls (nothing reads the result): bridge idle
        # windows so the pstate ramp does not reset.
        for i in range(n):
            jp = fu.tile([P, QC], f32, tag="f", name=f"junk_{tag_n}_{i}")
            nc.tensor.matmul(
                jp[:], junk_sb[0:P, 0:P], junk_sb[:, 0:QC],
                start=True, stop=True,
            )

    # ---- fill unit emitters ----
    def kt_unit(p2, mt2, pool, tg):
        kp = pool.tile([P, QC], f32, tag=tg, name=f"kp{p2}_{mt2}")
        for k in range(KT_TILES):
            nc.tensor.matmul(
                kp[:, 0:P],
                wk_sb[:, k, p2 * P:(p2 + 1) * P],
                ctx_sb[:, k, mt2 * P:(mt2 + 1) * P],
                start=(k == 0), stop=(k == KT_TILES - 1),
            )
        nc.vector.tensor_copy(kt_sb[p2][:, mt2 * P:(mt2 + 1) * P], kp[:, 0:P])

    def v_unit(half, mt2, pool, tg):
        vt = pool.tile([P, QC], f32, tag=tg, name=f"vt{half}_{mt2}")
        for k in range(KT_TILES):
            nc.tensor.matmul(
                vt[:, 0:P],
                ctx_sb[:, k, mt2 * P:(mt2 + 1) * P],
                wv_sb[:, k, half * P:(half + 1) * P],
                start=(k == 0), stop=(k == KT_TILES - 1),
            )
        nc.vector.tensor_copy(
            v_sb[:, mt2, 2 * half:2 * half + 2, 0:D],
            vt[:, 0:P].rearrange("p (h d) -> p h d", d=D),
        )

    qt_state = {}

    def qt_item(qc2, j, k):
        if k == 0:
            qt_state[(qc2, j)] = fg.tile([P, QC], f32, tag="g",
                                         name=f"qg{qc2}_{j}")
        t = qt_state[(qc2, j)]
        nc.tensor.matmul(
            t[:],
            wq_sb[:, k, j * P:(j + 1) * P],
            x_sb[:, k, qc2 * QC:(qc2 + 1) * QC],
            start=(k == 0), stop=(k == KT_TILES - 1),
        )
        if k == KT_TILES - 1:
            nc.vector.tensor_copy(qt_sb[j][:, qc2 * QC:(qc2 + 1) * QC], t[:])
            del qt_state[(qc2, j)]

    def fin_unit(nt_g, ec, ft=None, stage_act=False):
        if ft is None:
            ft = fu.tile([P, QC], f32, tag="f", name=f"fin{nt_g}_{ec}")
        for j in range(2):
            nc.tensor.matmul(
                ft[:],
                ao_sb[j][:, nt_g * P:(nt_g + 1) * P],
                wo_sb[:, j, ec * QC:(ec + 1) * QC],
                start=(j == 0), stop=(j == 1),
            )
        ost = outst_pool.tile([P, QC], bf16, tag="ostg", name=f"og{nt_g}_{ec}")
        if stage_act:
            nc.scalar.copy(ost[:], ft[:])
        else:
            nc.vector.tensor_copy(ost[:], ft[:])
        deng = nc.sync if (nt_g + ec) % 2 == 0 else nc.gpsimd
        deng.dma_start(out[nt_g * P:(nt_g + 1) * P, ec * QC:(ec + 1) * QC],
                       ost[:])

    # ---- attention O-chunk machinery (layout B, nt-major) ----
    es_tiles = {}
    o_state = {}
    norm_state = {}

    def o_drain(g2, c):
        qc2, p2 = divmod(g2, 2)
        nt_l, hh = divmod(c, 2)
        nt_g = qc2 * 4 + nt_l
        ot = o_state.pop((g2, c))
        if hh == 0:
            norm_state[(g2, nt_l)] = norm_pool.tile(
                [P, P], bf16, tag="nm", name=f"nm{g2}_{nt_l}")
        nm = norm_state[(g2, nt_l)]
        r = r_pool.tile([P, 1], f32, tag="r", name=f"r{g2}_{c}")
        nc.vector.reciprocal(r[:], ot[:, D:D + 1])
        nc.vector.tensor_scalar_mul(nm[:, hh * D:(hh + 1) * D], ot[:, 0:D], r[:])
        if hh == 1:
            tp = fu.tile([P, P], bf16, tag="f", name=f"tp{g2}_{nt_l}")
            nc.tensor.transpose(tp[:], nm[:], ident_sb[:])
            nc.vector.tensor_copy(ao_sb[p2][:, nt_g * P:(nt_g + 1) * P], tp[:])
            del norm_state[(g2, nt_l)]

    def o_chunk(g2, c):
        qc2, p2 = divmod(g2, 2)
        nt_l, hh = divmod(c, 2)
        h = 2 * p2 + hh
        ot = opool.tile([P, D + 1], f32, tag="o", name=f"o{g2}_{c}")
        for mt2 in range(MT):
            nc.tensor.matmul(
                ot[:, 0:D + 1],
                es_tiles[(g2, mt2)][:, hh * QC + nt_l * P:
                                    hh * QC + (nt_l + 1) * P],
                v_sb[:, mt2, h, :],
                start=(mt2 == 0), stop=(mt2 == MT - 1),
            )
        o_state[(g2, c)] = ot
        if c >= 1:
            o_drain(g2, c - 1)

    # ---- priority-class work queue ----
    # item: (cls, idx, cyc, min_gen, fn)
    queue = deque()

    def seed():
        def add(cls, idx, cyc, min_gen, fn):
            queue.append((cls, idx, cyc, min_gen, fn))

        def add_kt(cls, p2, mts):
            for i, mt2 in enumerate(mts):
                add(cls, mt2, 1024, 0, (lambda m=mt2, i2=i, pp=p2: kt_unit(
                    pp, m, opool if i2 % 2 == 0 else fu,
                    "o" if i2 % 2 == 0 else "f")))

        def add_v(cls, half, alt):
            for i, mt2 in enumerate(range(MT)):
                pool, tg = (opool, "o") if (alt and i % 2 == 0) else (fu, "f")
                add(cls, mt2, 1024, 0,
                    (lambda m=mt2, h2=half, pl=pool, t2=tg: v_unit(h2, m, pl, t2)))

        def add_o(cls, g2):
            for c in range(8):
                add(cls, c, 1100, g2 + 1, (lambda g3=g2, c2=c: o_chunk(g3, c2)))
            add(cls, 8, 150, g2 + 1, (lambda g3=g2: o_drain(g3, 7)))

        def add_qt(cls, qc2, j):
            for k in range(KT_TILES):
                add(cls, k, QC, 0, (lambda q=qc2, j2=j, k2=k: qt_item(q, j2, k2)))

        def add_fin(cls, qc2):
            for nt_l in range(4):
                for ec in range(2):
                    add(cls, nt_l * 2 + ec, 1024, 0,
                        (lambda n=qc2 * 4 + nt_l, e=ec: fin_unit(n, e)))

        add_kt(0, 0, range(3, MT))
        add_qt(1, 0, 1)
        add_kt(2, 1, range(MT))
        add_qt(3, 1, 0)
        add_qt(4, 1, 1)
        add_v(5, 0, True)
        add_o(6, 0)
        add_v(7, 1, False)
        add_qt(8, 2, 0)
        add_qt(9, 2, 1)
        add_o(10, 1)
        add_o(11, 2)
        add_fin(12, 0)
        add_qt(13, 3, 0)
        add_qt(14, 3, 1)
        add_o(15, 3)
        add_fin(16, 1)
        add_o(17, 4)
        add_o(18, 5)
        add_fin(19, 2)
        add_o(20, 6)

    seed()
    QT_CLS = {(0, 1): 1, (1, 0): 3, (1, 1): 4, (2, 0): 8, (2, 1): 9,
              (3, 0): 13, (3, 1): 14}
    O_CLS = {0: 6, 1: 10, 2: 11, 3: 15, 4: 17, 5: 18, 6: 20}
    cur_gen = [0]

    def drain_thru(cls_id):
        spent = 0
        while queue and queue[0][0] <= cls_id:
            cls, idx, cyc, mg, fn = queue.popleft()
            assert mg <= cur_gen[0], f"forced drain of blocked item {cls}/{idx}"
            fn()
            spent += cyc
        return spent

    def drain_units(cls_id, max_idx):
        spent = 0
        while queue and (queue[0][0] < cls_id
                         or (queue[0][0] == cls_id and queue[0][1] <= max_idx)):
            cls, idx, cyc, mg, fn = queue.popleft()
            assert mg <= cur_gen[0]
            fn()
            spent += cyc
        return spent

    def pump(budget):
        spent = 0
        while queue and spent < budget and queue[0][3] <= cur_gen[0]:
            cls, idx, cyc, mg, fn = queue.popleft()
            fn()
            spent += cyc
        return spent

    # ---- head: warmup + QT(qc0, j0) (the longer pole: x DMA + 8 mm +
    # drain gate the first s) then KT m0-2, via s-pool tiles ----
    emit_junk(HEAD_JUNK, "h")
    h0 = spool.tile([P, 2 * QC], f32, tag="s", name="h0")
    for k in range(KT_TILES):
        nc.tensor.matmul(
            h0[:, 0:QC],
            wq_sb[:, k, 0:P],
            x_sb[:, k, 0:QC],
            start=(k == 0), stop=(k == KT_TILES - 1),
        )
    emit_junk(MID_JUNK, "m")
    for k in range(KT_TILES):
        nc.tensor.matmul(
            h0[:, QC:QC + P],
            wk_sb[:, k, 0:P],
            ctx_sb[:, k, 0:P],
            start=(k == 0), stop=(k == KT_TILES - 1),
        )
    nc.vector.tensor_copy(qt_sb[0][:, 0:QC], h0[:, 0:QC])
    nc.vector.tensor_copy(kt_sb[0][:, 0:P], h0[:, QC:QC + P])

    # ---- main loop: 8 generations x 16 exp slots ----
    # s matmuls are emitted ONE slot ahead of their exp so the exp
    # stream never eats the s-completion sem latency.
    s_tiles = {}

    def emit_s(g2, mt2):
        qc2, p2 = divmod(g2, 2)
        s_t = spool.tile([P, 2 * QC], f32, tag="s", name=f"s{g2}_{mt2}")
        for hh in range(2):
            nc.tensor.matmul(
                s_t[:, hh * QC:(hh + 1) * QC],
                kt_sb[p2][hh * D:(hh + 1) * D, mt2 * P:(mt2 + 1) * P],
                qt_sb[p2][hh * D:(hh + 1) * D, qc2 * QC:(qc2 + 1) * QC],
                start=True, stop=True,
            )
        s_tiles[(g2, mt2)] = s_t

    h1 = spool.tile([P, 2 * QC], f32, tag="s", name="h1")
    for mloc, off in ((1, 0), (2, QC)):
        for k in range(KT_TILES):
            nc.tensor.matmul(
                h1[:, off:off + P],
                wk_sb[:, k, 0:P],
                ctx_sb[:, k, mloc * P:(mloc + 1) * P],
                start=(k == 0), stop=(k == KT_TILES - 1),
            )
    nc.vector.tensor_copy(kt_sb[0][:, P:2 * P], h1[:, 0:P])
    nc.vector.tensor_copy(kt_sb[0][:, 2 * P:3 * P], h1[:, QC:QC + P])
    emit_s(0, 0)
    for g in range(NGEN):
        cur_gen[0] = g
        qc, p = divmod(g, 2)
        for mt in range(MT):
            spent = 0
            if g == 0:
                spent += drain_units(0, min(mt + 1, MT - 1))
            elif g == 1:
                spent += drain_units(2, min(mt + 1, MT - 1))
            if g >= 2 and mt == 8:
                spent += drain_thru(O_CLS[g - 2])
            es_t = es_pool.tile([P, 2 * QC], bf16, tag="es", name=f"es{g}_{mt}")
            nc.scalar.activation(
                es_t[:], s_tiles.pop((g, mt)),
                mybir.ActivationFunctionType.Exp, scale=SCALE,
            )
            es_tiles[(g, mt)] = es_t
            if mt < MT - 1:
                emit_s(g, mt + 1)
            elif g + 1 < NGEN:
                if g + 1 == 1:
                    spent += drain_units(2, 0)
                elif g + 1 >= 2:
                    spent += drain_thru(QT_CLS[divmod(g + 1, 2)])
                emit_s(g + 1, 0)
            spent += 2 * QC
            pump(SLOT_BUDGET - spent)

    # ---- tail: O(gen 7) + output projection for qc3 ----
    # All chunks first (PE runs back-to-back), then the fins: keeps
    # DVE round-trips off the in-order PE queue's critical path. Tail
    # stage copies go to ACT (idle after the last exp).
    cur_gen[0] = NGEN
    drain_thru(20)
    g7 = NGEN - 1

    def tail_fins(nt_l):
        # tail fins use the (now idle) s-pool banks; stage copies split
        # across ACT (ec0) and DVE (ec1), both idle after the last exp.
        tt = spool.tile([P, 2 * QC], f32, tag="s", name=f"tfin{nt_l}")
        fin_unit(12 + nt_l, 0, ft=tt[:, 0:QC], stage_act=True)
        fin_unit(12 + nt_l, 1, ft=tt[:, QC:2 * QC], stage_act=False)

    fins_after = {2: 0, 4: 1, 6: 2}
    for c in range(8):
        o_chunk(g7, c)
        if c in fins_after:
            tail_fins(fins_after[c])
    o_drain(g7, 7)
    tail_fins(3)


def _build(reps=1, opt=None):
    key = (reps, tuple(sorted((opt or {}).items())))
    if key in _CACHE:
        return _CACHE[key]
    nc = bacc.Bacc("TRN2", target_bir_lowering=False, debug=False)
    xd = nc.dram_tensor("xd", [P, KT_TILES, N], bf16, kind="ExternalInput")
    ctxd = nc.dram_tensor("ctxd", [P, KT_TILES, M], bf16, kind="ExternalInput")
    wq = nc.dram_tensor("wq", [P, KT_TILES, IC], bf16, kind="ExternalInput")
    wk = nc.dram_tensor("wk", [P, KT_TILES, IC], bf16, kind="ExternalInput")
    wv = nc.dram_tensor("wv", [P, KT_TILES, IC], bf16, kind="ExternalInput")
    wo = nc.dram_tensor("wo", [P, 2, C], bf16, kind="ExternalInput")
    identd = nc.dram_tensor("ident", [P, P], bf16, kind="ExternalInput")
    out = nc.dram_tensor("out", [N, C], bf16, kind="ExternalOutput")
    with tile.TileContext(nc) as tc:
        for _ in range(reps):
            with ExitStack() as ctx:
                _body(nc, tc, ctx, xd, ctxd, wq, wk, wv, wo, identd, out,
                      opt=opt)
    nc.compile()
    _CACHE[key] = nc
    return nc


def _to_tiled(a, inner):
    """[K*128, inner] f32 -> [128, K, inner] bf16 (partition-major tiling)."""
    k = a.shape[0] // P
    return np.ascontiguousarray(
        a.reshape(k, P, inner).transpose(1, 0, 2).astype(ml_dtypes.bfloat16)
    )


def _shard_inputs(x, context, Wq, Wk, Wv, Wo):
    ident = np.eye(P, dtype=ml_dtypes.bfloat16)
    in_maps = []
    for c in range(NCORES):
        b, g = divmod(c, NCORES // B)
        cols = slice(g * IC, (g + 1) * IC)
        in_maps.append({
            "xd": _to_tiled(np.ascontiguousarray(x[b].T), N),
            "ctxd": _to_tiled(np.ascontiguousarray(context[b].T), M),
            "wq": _to_tiled(np.ascontiguousarray(Wq[:, cols]), IC),
            "wk": _to_tiled(np.ascontiguousarray(Wk[:, cols]), IC),
            "wv": _to_tiled(np.ascontiguousarray(Wv[:, cols]), IC),
            "wo": _to_tiled(np.ascontiguousarray(Wo[cols, :]), C),
            "ident": ident,
        })
    return in_maps


def kernel(x, context, Wq, Wk, Wv, Wo, reps=1):
    x = np.asarray(x, dtype=np.float32)
    context = np.asarray(context, dtype=np.float32)
    Wq, Wk, Wv, Wo = (np.asarray(w, dtype=np.float32) for w in (Wq, Wk, Wv, Wo))
    nc = _build(reps)
    in_maps = _shard_inputs(x, context, Wq, Wk, Wv, Wo)
    res = run_bass_kernel_spmd(nc, in_maps, core_ids=list(range(NCORES)))
    gpb = NCORES // B
    out = np.zeros((B, N, C), dtype=np.float32)
    for c in range(NCORES):
        out[c // gpb] += np.asarray(res.results[c]["out"], dtype=np.float32)
    return out


# revision 21
# speedup vs baseline: 1.0934x; 1.0099x over previous
"""Trainium2 Bass kernel for CrossAttention (B=2, N=M=2048, 16 heads x 64).

Sharding: batch x head-group parallel over 8 cores. Core c handles batch
c//4 and heads [4*(c%4), 4*(c%4)+4). Projection weights are column-split
(Wq/Wk/Wv) / row-split (Wo) per core; each core produces a partial
[2048, 1024] output (bf16) which the host sums per batch (4 partials).

V3 design (cost-model driven, all bf16):
  - Matmul cost = out_free_rows x cycles; contraction dim and output
    partition count are free.  The attn@V matmul therefore runs in
    "layout B": out[n 128, d 65] with lhsT = es[m, n-slice], rhs =
    v[m, 65] -- 66560 rows instead of 131072 (layout A).  Total PE:
    QKV proj 98304 + S 131072 + O 66560 + transpose 4096 + out-proj
    32768 = 332800 rows (138.7us floor at 2.4GHz).
  - es (exp of logits) persists in SBUF bf16 for 2.5 generations
    ((qc, head-pair) chunks); O accumulation is nt-major: each
    (n-tile, head) PSUM accumulator runs its 16 m-chunk matmuls
    back-to-back, so only 2 o-banks are live (PSUM: s 4 + fill-unit 1
    + fill-group 1 + o 2 = 8 banks).
  - Normalization: ones column in V gives denominators in o col 64;
    DVE reciprocal [P,1] + per-partition tensor_scalar_mul; PE
    transpose (identity matmul) flips [n, ic] -> [ic, n] for the
    output projection.
  - ACT runs only the softmax Exp (128 x [128,1024] = 132.9us).
  - Emission scheduling: a priority-class work queue (KT/QT/V/O/FIN
    units) pumped per exp-slot with a cycle budget plus forced drains
    at dependency barriers keeps PE saturated.
"""

import numpy as np
import ml_dtypes
from collections import deque
from contextlib import ExitStack

import concourse.tile as tile
from concourse import bacc, mybir
from concourse.bass_utils import run_bass_kernel_spmd

B, N, M, C = 2, 2048, 2048, 1024
HEADS, D = 16, 64
HPC = 4            # heads per core
IC = HPC * D       # 256 inner dims per core
SCALE = D ** -0.5
NCORES = 8
KT_TILES = C // 128   # 8 contraction tiles for projections
P = 128
MT = M // P           # 16 m tiles
QC = 512
NQC = N // QC         # 4 q chunks
NGEN = 2 * NQC        # 8 (qc, head-pair) generations
f32 = mybir.dt.float32
bf16 = mybir.dt.bfloat16

_CACHE = {}


def _body(nc, tc, ctx, xd, ctxd, wq, wk, wv, wo, identd, out, opt=None):
    opt = opt or {}
    SLOT_BUDGET = opt.get("slot_budget", 2500)
    HEAD_JUNK = opt.get("head_junk", 6)
    MID_JUNK = opt.get("mid_junk", 2)
    ES_BUFS = opt.get("es_bufs", 40)

    const = ctx.enter_context(tc.tile_pool(name="const", bufs=1))
    wq_sb = const.tile([P, KT_TILES, IC], bf16, tag="wq")
    wk_sb = const.tile([P, KT_TILES, IC], bf16, tag="wk")
    wv_sb = const.tile([P, KT_TILES, IC], bf16, tag="wv")
    wo_sb = const.tile([P, 2, C], bf16, tag="wo")
    ctx_sb = const.tile([P, KT_TILES, M], bf16, tag="ctx")
    x_sb = const.tile([P, KT_TILES, N], bf16, tag="x")
    kt_sb = [const.tile([P, M], bf16, tag=f"kt{j}", name=f"kt{j}") for j in range(2)]
    qt_sb = [const.tile([P, N], bf16, tag=f"qt{j}", name=f"qt{j}") for j in range(2)]
    v_sb = const.tile([P, MT, HPC, D + 1], bf16, tag="v")
    ao_sb = [const.tile([P, N], bf16, tag=f"ao{j}", name=f"ao{j}") for j in range(2)]
    ident_sb = const.tile([P, P], bf16, tag="ident")
    ones_sb = const.tile([P, 1], f32, tag="ones")
    junk_sb = const.tile([P, QC], bf16, tag="junk")

    es_pool = ctx.enter_context(tc.tile_pool(name="es", bufs=ES_BUFS))
    norm_pool = ctx.enter_context(tc.tile_pool(name="norm", bufs=2))
    r_pool = ctx.enter_context(tc.tile_pool(name="rp", bufs=4))
    outst_pool = ctx.enter_context(tc.tile_pool(name="outst", bufs=4))

    spool = ctx.enter_context(tc.tile_pool(name="s_ps", bufs=2, space="PSUM"))
    fu = ctx.enter_context(tc.tile_pool(name="fu_ps", bufs=1, space="PSUM"))
    fg = ctx.enter_context(tc.tile_pool(name="fg_ps", bufs=1, space="PSUM"))
    opool = ctx.enter_context(tc.tile_pool(name="o_ps", bufs=2, space="PSUM"))

    # junk tile first on DVE so PE warmup matmuls can start ~0.4us in
    nc.vector.memset(junk_sb[:], 0.0)
    nc.vector.memset(ones_sb[:], 1.0)
    # ones column of V_aug (denominator trick)
    nc.vector.tensor_copy(
        v_sb[:, :, :, D:D + 1],
        ones_sb[:, 0:1].to_broadcast((P, MT, HPC, 1)),
    )

    # ---- input DMAs ----
    # ACT must stay nearly DMA-free (exp starts ~8us and dma_start
    # occupies the issuing engine queue for the whole transfer): scalar
    # gets only wk; wq/wv ride the otherwise-idle DVE queue; x qc0
    # goes FIRST on SP/Pool (it gates the head QT -> first exp).
    nc.scalar.dma_start(wq_sb[:], wq[:])
    nc.scalar.dma_start(wk_sb[:], wk[:])
    for k in range(KT_TILES):
        eng = nc.sync if k < 4 else nc.gpsimd
        eng.dma_start(x_sb[:, k, 0:QC], xd[:, k, 0:QC])
    for k in range(KT_TILES):
        eng = nc.sync if k < 4 else nc.gpsimd
        eng.dma_start(ctx_sb[:, k, 0:QC], ctxd[:, k, 0:QC])
    for k in range(KT_TILES):
        eng = nc.sync if k % 2 == 0 else nc.gpsimd
        eng.dma_start(ctx_sb[:, k, QC:M], ctxd[:, k, QC:M])
    nc.sync.dma_start(wv_sb[:], wv[:])
    nc.gpsimd.dma_start(ident_sb[:], identd[:])
    nc.gpsimd.dma_start(wo_sb[:], wo[:])
    for k in range(KT_TILES):
        eng = nc.sync if k % 2 == 0 else nc.gpsimd
        eng.dma_start(x_sb[:, k, QC:N], xd[:, k, QC:N])
    # preload the Exp activation table so exp(0) doesn't pay ATL
    nc.scalar.activation(ones_sb[:], ones_sb[:],
                         mybir.ActivationFunctionType.Exp)

    def emit_junk(n, tag_n):
        # PE keep-warm matmuls (nothing reads the result): bridge idle
        # windows so the pstate ramp does not reset.
        for i in range(n):
            jp = fu.tile([P, QC], f32, tag="f", name=f"junk_{tag_n}_{i}")
            nc.tensor.matmul(
                jp[:], junk_sb[0:P, 0:P], junk_sb[:, 0:QC],
                start=True, stop=True,
            )

    # ---- fill unit emitters ----
    def kt_unit(p2, mt2, pool, tg):
        kp = pool.tile([P, QC], f32, tag=tg, name=f"kp{p2}_{mt2}")
        for k in range(KT_TILES):
            nc.tensor.matmul(
                kp[:, 0:P],
                wk_sb[:, k, p2 * P:(p2 + 1) * P],
                ctx_sb[:, k, mt2 * P:(mt2 + 1) * P],
                start=(k == 0), stop=(k == KT_TILES - 1),
            )
        nc.vector.tensor_copy(kt_sb[p2][:, mt2 * P:(mt2 + 1) * P], kp[:, 0:P])

    def v_unit(half, mt2, pool, tg):
        vt = pool.tile([P, QC], f32, tag=tg, name=f"vt{half}_{mt2}")
        for k in range(KT_TILES):
            nc.tensor.matmul(
                vt[:, 0:P],
                ctx_sb[:, k, mt2 * P:(mt2 + 1) * P],
                wv_sb[:, k, half * P:(half + 1) * P],
                start=(k == 0), stop=(k == KT_TILES - 1),
            )
        nc.vector.tensor_copy(
            v_sb[:, mt2, 2 * half:2 * half + 2, 0:D],
            vt[:, 0:P].rearrange("p (h d) -> p h d", d=D),
        )

    qt_state = {}

    def qt_item(qc2, j, k):
        if k == 0:
            qt_state[(qc2, j)] = fg.tile([P, QC], f32, tag="g",
                                         name=f"qg{qc2}_{j}")
        t = qt_state[(qc2, j)]
        nc.tensor.matmul(
            t[:],
            wq_sb[:, k, j * P:(j + 1) * P],
            x_sb[:, k, qc2 * QC:(qc2 + 1) * QC],
            start=(k == 0), stop=(k == KT_TILES - 1),
        )
        if k == KT_TILES - 1:
            nc.vector.tensor_copy(qt_sb[j][:, qc2 * QC:(qc2 + 1) * QC], t[:])
            del qt_state[(qc2, j)]

    def fin_unit(nt_g, ec, ft=None, stage_act=False):
        if ft is None:
            ft = fu.tile([P, QC], f32, tag="f", name=f"fin{nt_g}_{ec}")
        for j in range(2):
            nc.tensor.matmul(
                ft[:],
                ao_sb[j][:, nt_g * P:(nt_g + 1) * P],
                wo_sb[:, j, ec * QC:(ec + 1) * QC],
                start=(j == 0), stop=(j == 1),
            )
        ost = outst_pool.tile([P, QC], bf16, tag="ostg", name=f"og{nt_g}_{ec}")
        if stage_act:
            nc.scalar.copy(ost[:], ft[:])
        else:
            nc.vector.tensor_copy(ost[:], ft[:])
        deng = nc.sync if (nt_g + ec) % 2 == 0 else nc.gpsimd
        deng.dma_start(out[nt_g * P:(nt_g + 1) * P, ec * QC:(ec + 1) * QC],
                       ost[:])

    # ---- attention O-chunk machinery (layout B, nt-major) ----
    es_tiles = {}
    o_state = {}
    norm_state = {}

    def o_drain(g2, c):
        qc2, p2 = divmod(g2, 2)
        nt_l, hh = divmod(c, 2)
        nt_g = qc2 * 4 + nt_l
        ot = o_state.pop((g2, c))
        if hh == 0:
            norm_state[(g2, nt_l)] = norm_pool.tile(
                [P, P], bf16, tag="nm", name=f"nm{g2}_{nt_l}")
        nm = norm_state[(g2, nt_l)]
        r = r_pool.tile([P, 1], f32, tag="r", name=f"r{g2}_{c}")
        nc.vector.reciprocal(r[:], ot[:, D:D + 1])
        nc.vector.tensor_scalar_mul(nm[:, hh * D:(hh + 1) * D], ot[:, 0:D], r[:])
        if hh == 1:
            tp = fu.tile([P, P], bf16, tag="f", name=f"tp{g2}_{nt_l}")
            nc.tensor.transpose(tp[:], nm[:], ident_sb[:])
            nc.vector.tensor_copy(ao_sb[p2][:, nt_g * P:(nt_g + 1) * P], tp[:])
            del norm_state[(g2, nt_l)]

    def o_chunk(g2, c):
        qc2, p2 = divmod(g2, 2)
        nt_l, hh = divmod(c, 2)
        h = 2 * p2 + hh
        ot = opool.tile([P, D + 1], f32, tag="o", name=f"o{g2}_{c}")
        for mt2 in range(MT):
            nc.tensor.matmul(
                ot[:, 0:D + 1],
                es_tiles[(g2, mt2)][:, hh * QC + nt_l * P:
                                    hh * QC + (nt_l + 1) * P],
                v_sb[:, mt2, h, :],
                start=(mt2 == 0), stop=(mt2 == MT - 1),
            )
        o_state[(g2, c)] = ot
        if c >= 1:
            o_drain(g2, c - 1)

    # ---- priority-class work queue ----
    # item: (cls, idx, cyc, min_gen, fn)
    queue = deque()

    def seed():
        def add(cls, idx, cyc, min_gen, fn):
            queue.append((cls, idx, cyc, min_gen, fn))

        def add_kt(cls, p2, mts):
            for i, mt2 in enumerate(mts):
                add(cls, mt2, 1024, 0, (lambda m=mt2, i2=i, pp=p2: kt_unit(
                    pp, m, opool if i2 % 2 == 0 else fu,
                    "o" if i2 % 2 == 0 else "f")))

        def add_v(cls, half, alt):
            for i, mt2 in enumerate(range(MT)):
                pool, tg = (opool, "o") if (alt and i % 2 == 0) else (fu, "f")
                add(cls, mt2, 1024, 0,
                    (lambda m=mt2, h2=half, pl=pool, t2=tg: v_unit(h2, m, pl, t2)))

        def add_o(cls, g2):
            for c in range(8):
                add(cls, c, 1100, g2 + 1, (lambda g3=g2, c2=c: o_chunk(g3, c2)))
            add(cls, 8, 150, g2 + 1, (lambda g3=g2: o_drain(g3, 7)))

        def add_qt(cls, qc2, j):
            for k in range(KT_TILES):
                add(cls, k, QC, 0, (lambda q=qc2, j2=j, k2=k: qt_item(q, j2, k2)))

        def add_fin(cls, qc2):
            for nt_l in range(4):
                for ec in range(2):
                    add(cls, nt_l * 2 + ec, 1024, 0,
                        (lambda n=qc2 * 4 + nt_l, e=ec: fin_unit(n, e)))

        add_kt(0, 0, range(1, MT))
        add_qt(1, 0, 1)
        add_kt(2, 1, range(MT))
        add_qt(3, 1, 0)
        add_qt(4, 1, 1)
        add_v(5, 0, True)
        add_o(6, 0)
        add_v(7, 1, False)
        add_qt(8, 2, 0)
        add_qt(9, 2, 1)
        add_o(10, 1)
        add_o(11, 2)
        add_fin(12, 0)
        add_qt(13, 3, 0)
        add_qt(14, 3, 1)
        add_o(15, 3)
        add_fin(16, 1)
        add_o(17, 4)
        add_o(18, 5)
        add_fin(19, 2)
        add_o(20, 6)

    seed()
    QT_CLS = {(0, 1): 1, (1, 0): 3, (1, 1): 4, (2, 0): 8, (2, 1): 9,
              (3, 0): 13, (3, 1): 14}
    O_CLS = {0: 6, 1: 10, 2: 11, 3: 15, 4: 17, 5: 18, 6: 20}
    cur_gen = [0]

    def drain_thru(cls_id):
        spent = 0
        while queue and queue[0][0] <= cls_id:
            cls, idx, cyc, mg, fn = queue.popleft()
            assert mg <= cur_gen[0], f"forced drain of blocked item {cls}/{idx}"
            fn()
            spent += cyc
        return spent

    def drain_units(cls_id, max_idx):
        spent = 0
        while queue and (queue[0][0] < cls_id
                         or (queue[0][0] == cls_id and queue[0][1] <= max_idx)):
            cls, idx, cyc, mg, fn = queue.popleft()
            assert mg <= cur_gen[0]
            fn()
            spent += cyc
        return spent

    def pump(budget):
        spent = 0
        while queue and spent < budget and queue[0][3] <= cur_gen[0]:
            cls, idx, cyc, mg, fn = queue.popleft()
            fn()
            spent += cyc
        return spent

    # ---- head: warmup + QT(qc0, j0) (the longer pole: x DMA + 8 mm +
    # drain gate the first s) then KT m0, via separate s-pool tiles
    # (tile-granular deps would otherwise delay the qt drain) ----
    emit_junk(HEAD_JUNK, "h")
    h0 = spool.tile([P, 2 * QC], f32, tag="s", name="h0")
    for k in range(KT_TILES):
        nc.tensor.matmul(
            h0[:, 0:QC],
            wq_sb[:, k, 0:P],
            x_sb[:, k, 0:QC],
            start=(k == 0), stop=(k == KT_TILES - 1),
        )
    nc.vector.tensor_copy(qt_sb[0][:, 0:QC], h0[:, 0:QC])

    # ---- main loop: 8 generations x 16 exp slots ----
    # s matmuls are emitted ONE slot ahead of their exp so the exp
    # stream never eats the s-completion sem latency.
    s_tiles = {}

    def emit_s(g2, mt2):
        qc2, p2 = divmod(g2, 2)
        s_t = spool.tile([P, 2 * QC], f32, tag="s", name=f"s{g2}_{mt2}")
        for hh in range(2):
            nc.tensor.matmul(
                s_t[:, hh * QC:(hh + 1) * QC],
                kt_sb[p2][hh * D:(hh + 1) * D, mt2 * P:(mt2 + 1) * P],
                qt_sb[p2][hh * D:(hh + 1) * D, qc2 * QC:(qc2 + 1) * QC],
                start=True, stop=True,
            )
        s_tiles[(g2, mt2)] = s_t

    h1 = spool.tile([P, 2 * QC], f32, tag="s", name="h1")
    for k in range(KT_TILES):
        nc.tensor.matmul(
            h1[:, 0:P],
            wk_sb[:, k, 0:P],
            ctx_sb[:, k, 0:P],
            start=(k == 0), stop=(k == KT_TILES - 1),
        )
    nc.vector.tensor_copy(kt_sb[0][:, 0:P], h1[:, 0:P])
    emit_s(0, 0)
    for g in range(NGEN):
        cur_gen[0] = g
        qc, p = divmod(g, 2)
        for mt in range(MT):
            spent = 0
            if g == 0:
                spent += drain_units(0, min(mt + 1, MT - 1))
            elif g == 1:
                spent += drain_units(2, min(mt + 1, MT - 1))
            if g >= 2 and mt == 8:
                spent += drain_thru(O_CLS[g - 2])
            es_t = es_pool.tile([P, 2 * QC], bf16, tag="es", name=f"es{g}_{mt}")
            nc.scalar.activation(
                es_t[:], s_tiles.pop((g, mt)),
                mybir.ActivationFunctionType.Exp, scale=SCALE,
            )
            es_tiles[(g, mt)] = es_t
            if mt < MT - 1:
                emit_s(g, mt + 1)
            elif g + 1 < NGEN:
                if g + 1 == 1:
                    spent += drain_units(2, 0)
                elif g + 1 >= 2:
                    spent += drain_thru(QT_CLS[divmod(g + 1, 2)])
                emit_s(g + 1, 0)
            spent += 2 * QC
            pump(SLOT_BUDGET - spent)

    # ---- tail: O(gen 7) + output projection for qc3 ----
    # All chunks first (PE runs back-to-back), then the fins: keeps
    # DVE round-trips off the in-order PE queue's critical path. Tail
    # stage copies go to ACT (idle after the last exp).
    cur_gen[0] = NGEN
    drain_thru(20)
    g7 = NGEN - 1

    def tail_fins(nt_l):
        # tail fins use the (now idle) s-pool banks; stage copies split
        # across ACT (ec0) and DVE (ec1), both idle after the last exp.
        tt = spool.tile([P, 2 * QC], f32, tag="s", name=f"tfin{nt_l}")
        fin_unit(12 + nt_l, 0, ft=tt[:, 0:QC], stage_act=True)
        fin_unit(12 + nt_l, 1, ft=tt[:, QC:2 * QC], stage_act=False)

    fins_after = {2: 0, 4: 1, 6: 2}
    for c in range(8):
        o_chunk(g7, c)
        if c in fins_after:
            tail_fins(fins_after[c])
    o_drain(g7, 7)
    tail_fins(3)


def _build(reps=1, opt=None):
    key = (reps, tuple(sorted((opt or {}).items())))
    if key in _CACHE:
        return _CACHE[key]
    nc = bacc.Bacc("TRN2", target_bir_lowering=False, debug=False)
    xd = nc.dram_tensor("xd", [P, KT_TILES, N], bf16, kind="ExternalInput")
    ctxd = nc.dram_tensor("ctxd", [P, KT_TILES, M], bf16, kind="ExternalInput")
    wq = nc.dram_tensor("wq", [P, KT_TILES, IC], bf16, kind="ExternalInput")
    wk = nc.dram_tensor("wk", [P, KT_TILES, IC], bf16, kind="ExternalInput")
    wv = nc.dram_tensor("wv", [P, KT_TILES, IC], bf16, kind="ExternalInput")
    wo = nc.dram_tensor("wo", [P, 2, C], bf16, kind="ExternalInput")
    identd = nc.dram_tensor("ident", [P, P], bf16, kind="ExternalInput")
    out = nc.dram_tensor("out", [N, C], bf16, kind="ExternalOutput")
    with tile.TileContext(nc) as tc:
        for _ in range(reps):
            with ExitStack() as ctx:
                _body(nc, tc, ctx, xd, ctxd, wq, wk, wv, wo, identd, out,
                      opt=opt)
    nc.compile()
    _CACHE[key] = nc
    return nc


def _to_tiled(a, inner):
    """[K*128, inner] f32 -> [128, K, inner] bf16 (partition-major tiling)."""
    k = a.shape[0] // P
    return np.ascontiguousarray(
        a.reshape(k, P, inner).transpose(1, 0, 2).astype(ml_dtypes.bfloat16)
    )


def _shard_inputs(x, context, Wq, Wk, Wv, Wo):
    ident = np.eye(P, dtype=ml_dtypes.bfloat16)
    in_maps = []
    for c in range(NCORES):
        b, g = divmod(c, NCORES // B)
        cols = slice(g * IC, (g + 1) * IC)
        in_maps.append({
            "xd": _to_tiled(np.ascontiguousarray(x[b].T), N),
            "ctxd": _to_tiled(np.ascontiguousarray(context[b].T), M),
            "wq": _to_tiled(np.ascontiguousarray(Wq[:, cols]), IC),
            "wk": _to_tiled(np.ascontiguousarray(Wk[:, cols]), IC),
            "wv": _to_tiled(np.ascontiguousarray(Wv[:, cols]), IC),
            "wo": _to_tiled(np.ascontiguousarray(Wo[cols, :]), C),
            "ident": ident,
        })
    return in_maps


def kernel(x, context, Wq, Wk, Wv, Wo, reps=1):
    x = np.asarray(x, dtype=np.float32)
    context = np.asarray(context, dtype=np.float32)
    Wq, Wk, Wv, Wo = (np.asarray(w, dtype=np.float32) for w in (Wq, Wk, Wv, Wo))
    nc = _build(reps)
    in_maps = _shard_inputs(x, context, Wq, Wk, Wv, Wo)
    res = run_bass_kernel_spmd(nc, in_maps, core_ids=list(range(NCORES)))
    gpb = NCORES // B
    out = np.zeros((B, N, C), dtype=np.float32)
    for c in range(NCORES):
        out[c // gpb] += np.asarray(res.results[c]["out"], dtype=np.float32)
    return out
